# revision 60
# baseline (speedup 1.0000x reference)
"""Multi-head attention block on 8 TRN2 NeuronCores.

Problem: x[2,2048,768] -> qkv proj -> 12-head attention -> out proj.
Sharding: 24 (batch, head) pairs across 8 cores; core c handles batch
c//4 and heads 3*(c%4)..3*(c%4)+2. Each core computes its heads'
Q,K,V, attention, and a partial output projection; the host sums the
four per-batch partials and adds the bias terms.

Design notes (v8, ~169us vs v6's ~176us):
  - Exp split across engines: odd key chunks compute exp on the DVE
    via a Schraudolph bit trick (one tensor_scalar:
    int16(s*184.665 + 16249) bitcast to bf16; the ~3% elementwise
    error mostly cancels in softmax, rel err 1.07e-2 vs the 2e-2
    gate). This halves the ACT exp stream, making the attention
    inner loop PE-bound (~98% TensorE occupancy) instead of
    ACT-bound. Pair-end raw copies move to ACT for the same reason.
  - One PSUM pool for the whole kernel: s tiles (3 bufs, 6 banks) +
    two AV accumulators (2 banks). Front work (QKV projections, V
    chunks) and the output projection borrow s slots, so they can
    interleave anywhere in the attention stream.
  - Minimal pre-pair front: only kh2/qB over the first x^T halves
    runs before attention; kh0/kh1, the b2/b3 projections and all V
    chunks ride inside pair (2,0) while the second x^T halves
    stream in (separate first/second-half x^T tiles keep the
    dependencies decoupled). Attention starts ~33us in (was ~47us).
  - Pair order (2,0),(0,0),(1,0),(2,1),(0,1),(1,1); the block-0
    output projection rides inside the block-1 pairs, and the tail
    runs the ah2[1] halves of the first three remaining proj chunks
    under the last flush's normalize chain (proj accumulates head 2
    first for this reason). Output DMA alternates sync/scalar
    queues so the final drain is not serialized.
  - All matmul operands bf16; output bf16; host sums partials in
    f32. K/Q fused weight stream, packed Q tiles, zero-carrying K
    tiles, ones-column V tiles for the softmax sums, DVE-memset
    constants and PE warmup matmuls carried over from v6.
  - Things measured NOT to help on this hardware: tile_position row
    tiling for the 64-deep QK contraction (no MM concurrency, and
    half-array matmuls re-throttle the HAM clock gate to 1.2GHz),
    1024-column moving matmuls (ISA caps at 512), xt DMA on the
    gpsimd queue, gpsimd tensor ops beyond partition_broadcast
    (multi-us library reloads), fp8 DoubleRow AV (needs fp8 exp
    output, which blows the error budget).
"""

import os
import sys

for _p in ("/opt/trn_rl_repo", "/opt/pypackages"):
    if _p not in sys.path:
        sys.path.append(_p)

import numpy as np

B, N, C = 2, 2048, 768
H, D = 12, 64
HPC = 3                    # heads per core
J = HPC * D                # 192 per-core head-dim rows
NCORES = 8
NBLK = 1024                # query-block width (one exp per [128, NBLK])
NB = N // NBLK             # 2
MC = N // 128              # 16 key chunks
KC = C // 128              # 6 contraction chunks for projections
NWARM = 12

SCH_A = 184.6649652337873   # 2^7 / ln 2
SCH_B = 16249.0             # exponent-bias offset, tuned for softmax

_cache = {}
LAST_RESULTS = None


def _build():
    import concourse.mybir as mybir
    import concourse.tile as tile
    from concourse import bacc

    f32 = mybir.dt.float32
    bf16 = mybir.dt.bfloat16
    i16 = mybir.dt.int16
    Exp = mybir.ActivationFunctionType.Exp
    Copy = mybir.ActivationFunctionType.Copy
    mult = mybir.AluOpType.mult
    add = mybir.AluOpType.add

    nc = bacc.Bacc("TRN2", target_bir_lowering=False, debug=False,
                   num_devices=NCORES)

    xt_d = nc.declare_dram_parameter("xt", [C, N], bf16, isOutput=False)
    # fused [K_h0|K_h1|K_h2|Q_h0|Q_h1|Q_h2] weight columns
    wkq_d = nc.declare_dram_parameter("wkq", [C, 2 * J], bf16,
                                      isOutput=False)
    wv_d = nc.declare_dram_parameter("wv", [C, J], bf16, isOutput=False)
    bq_d = nc.declare_dram_parameter("bq", [J, 1], f32, isOutput=False)
    # padded proj weights: rows 0:128 = heads 0,1; 128:192 = head 2;
    # 192:256 = zero (annihilates ah2[1]'s junk bottom half)
    wp_d = nc.declare_dram_parameter("wp", [2 * 128, C], bf16,
                                     isOutput=False)
    out_d = nc.declare_dram_parameter("out", [N, C], bf16, isOutput=True)

    with tile.TileContext(nc) as tc:
        with (
            tc.tile_pool(name="persist", bufs=1) as pp,
            tc.tile_pool(name="osb", bufs=4) as posb,
            tc.tile_pool(name="etile", bufs=6) as pe,
            tc.tile_pool(name="bcsb", bufs=2) as pbc,
        ):
            warm_t = pp.tile([128, 512], bf16, tag="warm_t", name="warm_t")
            wkq = [pp.tile([128, 2 * J], bf16, tag=f"wkq{k}",
                           name=f"wkq{k}") for k in range(KC)]
            # x^T halves as separate tiles so first-half
            # consumers never wait on the second-half DMA
            xtf = [pp.tile([128, 1024], bf16, tag=f"xtf{k}",
                           name=f"xtf{k}") for k in range(KC)]
            xts = [pp.tile([128, 1024], bf16, tag=f"xts{k}",
                           name=f"xts{k}") for k in range(KC)]
            wv = [pp.tile([128, J], bf16, tag=f"wv{k}", name=f"wv{k}")
                  for k in range(KC)]
            bqt = [pp.tile([64, 1], f32, tag=f"bq{h}", name=f"bq{h}")
                   for h in range(HPC)]
            wp = [pp.tile([128, C], bf16, tag=f"wp{t}", name=f"wp{t}")
                  for t in range(2)]
            # K^T per head, zero rows alternating so the packed Q tiles
            # need none: kh0 data 0:64, kh1 data 64:128, kh2 data 0:64
            kh = [pp.tile([128, N], bf16, tag=f"kh{h}", name=f"kh{h}")
                  for h in range(HPC)]
            # Q^T packed: qA = (q0 top, q1 bottom); qB = (q2 top, junk)
            qA = pp.tile([128, N], bf16, tag="qA", name="qA")
            qB = pp.tile([128, N], bf16, tag="qB", name="qB")
            # V with a ones column per head: [128, 3*65]
            vx = [pp.tile([128, HPC * 65], bf16, tag=f"vx{m}",
                          name=f"vx{m}") for m in range(MC)]
            sums = [pp.tile([1, N], f32, tag=f"sums{h}", name=f"sums{h}")
                    for h in range(HPC)]
            # unnormalized attention outputs (release AV PSUM fast),
            # base partition 0 so tensor_mul's SBUF inputs stay aligned
            raw = [pp.tile([64, N], bf16, tag=f"raw{h}", name=f"raw{h}")
                   for h in range(HPC)]
            ah2 = [pp.tile([128, N], bf16, tag=f"ah2{t}", name=f"ah2{t}")
                   for t in range(2)]

            # ---- constants via DVE memset (no DMA traffic; gpsimd
            # memset on partition-offset APs crashed the device) ----
            nc.vector.memset(warm_t[:], 1.0)
            nc.vector.memset(kh[0][64:128, :], 0.0)
            nc.vector.memset(kh[1][0:64, :], 0.0)
            nc.vector.memset(kh[2][64:128, :], 0.0)
            nc.vector.memset(qB[64:128, :], 0.0)
            nc.vector.memset(ah2[1][64:128, :], 0.0)
            for m in range(MC):
                on = vx[m].rearrange("p (h e) -> p h e", e=65)[:, :, 64:65]
                nc.vector.memset(on, 1.0)

            # ---- input DMA, ordered for earliest compute start ----
            # sync queue: x^T half-row tiles only (the long pole; each
            # extra trigger ahead of an xt tile delays it ~0.6us), first
            # halves feed blocks 0/1, then second halves
            for k in range(KC):
                nc.sync.dma_start(xtf[k][:],
                                  xt_d[128 * k:128 * (k + 1), 0:1024])
            for k in range(KC):
                nc.sync.dma_start(xts[k][:],
                                  xt_d[128 * k:128 * (k + 1), 1024:2048])
            # scalar queue (ACT idle early): fused KQ weights first
            # (k-th tile lands just ahead of the k-th accumulation
            # step; the tiny bias tiles would delay wkq[0] ~0.6us per
            # trigger), then q-bias, then V weights
            for k in range(KC):
                nc.scalar.dma_start(wkq[k][:], wkq_d[128 * k:128 * (k + 1), :])
            for h in range(HPC):
                nc.scalar.dma_start(bqt[h][:], bq_d[64 * h:64 * (h + 1), :])
            for k in range(KC):
                nc.scalar.dma_start(wv[k][:], wv_d[128 * k:128 * (k + 1), :])

            # wp is only needed by the projection tail; queue it after
            # the x^T burst
            for t in range(2):
                nc.sync.dma_start(wp[t][:], wp_d[128 * t:128 * (t + 1), :])

            with tc.tile_pool(name="ps2", bufs=1, space="PSUM") as ps2:
                # all front work (warmup, QKV projections, V chunks)
                # borrows "s" PSUM slots, so half of it can interleave
                # into the first attention pair while the second x^T
                # halves are still streaming in
                for i in range(NWARM):
                    ps = ps2.tile([128, NBLK], f32, tag="s", bufs=3,
                                  name=f"warm{i}")
                    nc.tensor.matmul(ps[:, 0:512], warm_t[:, 0:128],
                                     warm_t[:])

                def g_group(b, g):
                    # g0: K_h0,K_h1 / g1: K_h2,Q_h0 / g2: Q_h1,Q_h2
                    nsl = slice(512 * b, 512 * (b + 1))
                    xh = xtf if b < 2 else xts
                    hsl = slice(512 * (b % 2), 512 * (b % 2 + 1))
                    pf = ps2.tile([128, NBLK], f32, tag="s", bufs=3,
                                  name="ps_qk")
                    ps = pf[:, 0:512]
                    for k in range(KC):
                        nc.tensor.matmul(
                            ps, wkq[k][:, 128 * g:128 * (g + 1)],
                            xh[k][:, hsl],
                            start=(k == 0), stop=(k == KC - 1))
                    if g == 0:
                        nc.vector.tensor_copy(kh[0][0:64, nsl], ps[0:64, :])
                        nc.vector.tensor_copy(kh[1][64:128, nsl],
                                              ps[64:128, :])
                    elif g == 1:
                        nc.vector.tensor_copy(kh[2][0:64, nsl], ps[0:64, :])
                        nc.vector.tensor_scalar(
                            qA[0:64, nsl], ps[64:128, :], 0.125,
                            bqt[0][:], mult, add)
                    else:
                        nc.vector.tensor_scalar(
                            qA[64:128, nsl], ps[0:64, :], 0.125,
                            bqt[1][:], mult, add)
                        nc.vector.tensor_scalar(
                            qB[0:64, nsl], ps[64:128, :], 0.125,
                            bqt[2][:], mult, add)

                def v_chunk(m):
                    xh = xtf if m < 8 else xts
                    msl = slice(128 * (m % 8), 128 * (m % 8 + 1))
                    pf = ps2.tile([128, NBLK], f32, tag="s", bufs=3,
                                  name="ps_v")
                    ps = pf[:, 0:512]
                    for k in range(KC):
                        nc.tensor.matmul(ps[:, 0:J], xh[k][:, msl], wv[k][:],
                                         start=(k == 0), stop=(k == KC - 1))
                    vdst = vx[m].rearrange("p (h e) -> p h e",
                                           e=65)[:, :, 0:64]
                    nc.vector.tensor_copy(
                        vdst, ps[:, 0:J].rearrange("p (h e) -> p h e", e=64))

                pend = []

                def av_flush():
                    avh, h, nb, m, e = pend.pop(0)
                    vsl = slice(65 * h, 65 * (h + 1))
                    for i in range(2):
                        nc.tensor.matmul(
                            avh[i][:], vx[m][:, vsl],
                            e[:, 512 * i:512 * (i + 1)],
                            start=(m == 0), stop=(m == MC - 1))
                    if m != MC - 1:
                        return
                    adst, r0 = ((ah2[0], 0) if h == 0 else
                                (ah2[0], 64) if h == 1 else
                                (ah2[1], 0))
                    # raw copies first — they alone gate the next pair's
                    # AV PSUM writes; on ACT so the DVE exp stream is
                    # not disturbed. Sums rows and the broadcast/
                    # reciprocal/multiply chains follow off the critical
                    # path
                    for i in range(2):
                        hf = slice(NBLK * nb + 512 * i,
                                   NBLK * nb + 512 * (i + 1))
                        nc.scalar.activation(raw[h][:, hf],
                                             avh[i][0:64, :], Copy)
                    for i in range(2):
                        hf = slice(NBLK * nb + 512 * i,
                                   NBLK * nb + 512 * (i + 1))
                        nc.vector.tensor_copy(sums[h][:, hf],
                                              avh[i][64:65, :])
                    for i in range(2):
                        hf = slice(NBLK * nb + 512 * i,
                                   NBLK * nb + 512 * (i + 1))
                        bcs = pbc.tile([64, 512], f32, tag="bcs",
                                       name="bcs")
                        nc.gpsimd.partition_broadcast(bcs[:],
                                                      sums[h][:, hf])
                        rec = pbc.tile([64, 512], f32, tag="rec",
                                       name="rec")
                        nc.vector.reciprocal_approx_fast(rec[:], bcs[:])
                        nc.vector.tensor_mul(adst[r0:r0 + 64, hf],
                                             raw[h][:, hf], rec[:])

                def attn_pair(h, nb, extra=None):
                    # extra: dict chunk -> [thunks] fired after that
                    # chunk (interleaved projection / late front work)
                    qt = qA if h < 2 else qB
                    avh = [ps2.tile([65, 512], f32, tag=f"av{i}", bufs=1,
                                    name=f"ps_av{i}") for i in range(2)]
                    for m in range(MC):
                        msl = slice(128 * m, 128 * (m + 1))
                        s = ps2.tile([128, NBLK], f32, tag="s", bufs=3,
                                     name="ps_s")
                        for i in range(2):
                            q0 = NBLK * nb + 512 * i
                            nc.tensor.matmul(
                                s[:, 512 * i:512 * (i + 1)],
                                kh[h][:, msl], qt[:, q0:q0 + 512])
                        e = pe.tile([128, NBLK], bf16, tag="e", name="e")
                        if m % 2 == 1:
                            # odd chunks: Schraudolph exp on DVE
                            # (int16 bit trick, ~3% elementwise error
                            # that mostly cancels in softmax) to keep
                            # the ACT engine off the critical path
                            nc.vector.tensor_scalar(
                                e.bitcast(i16)[:], s[:],
                                SCH_A, SCH_B, mult, add)
                        else:
                            nc.scalar.activation(e[:], s[:], Exp)
                        pend.append((avh, h, nb, m, e))
                        if len(pend) > 2:
                            av_flush()
                        if extra and m in extra:
                            for th in extra[m]:
                                th()

                def proj(m):
                    # output projection chunk, borrowing an "s" PSUM
                    # slot so it can interleave into attention pairs;
                    # PSUM evacuation split across ACT and DVE
                    msl = slice(128 * m, 128 * (m + 1))
                    proj1(m)
                    proj2(m)

                pjt = {}

                def proj1(m):
                    # ah2[1] (head 2) part first: it is flushed two
                    # pairs before ah2[0], so these matmuls can run
                    # while the last pair's normalize chain completes
                    msl = slice(128 * m, 128 * (m + 1))
                    t = ps2.tile([128, NBLK], f32, tag="s", bufs=3,
                                 name="ps_pj")
                    pjt[m] = t
                    nc.tensor.matmul(t[:, 0:512], ah2[1][:, msl],
                                     wp[1][:, 0:512],
                                     start=True, stop=False)
                    nc.tensor.matmul(t[:, 512:768], ah2[1][:, msl],
                                     wp[1][:, 512:768],
                                     start=True, stop=False)

                def proj2(m):
                    msl = slice(128 * m, 128 * (m + 1))
                    t = pjt.pop(m)
                    nc.tensor.matmul(t[:, 0:512], ah2[0][:, msl],
                                     wp[0][:, 0:512],
                                     start=False, stop=True)
                    nc.tensor.matmul(t[:, 512:768], ah2[0][:, msl],
                                     wp[0][:, 512:768],
                                     start=False, stop=True)
                    o3 = posb.tile([128, C], bf16, tag="o3", name="o3")
                    nc.vector.tensor_copy(o3[:, 0:512], t[:, 0:512])
                    nc.scalar.activation(o3[:, 512:768], t[:, 512:768],
                                         Copy)
                    # alternate output queues so the final DMA drain
                    # is not serialized on one ring
                    oq = nc.sync if m % 2 == 0 else nc.scalar
                    oq.dma_start(out_d[msl, :], o3[:])

                def gg(b, g):
                    return lambda: g_group(b, g)

                def vv(m):
                    return lambda: v_chunk(m)

                def pj(m):
                    return lambda: proj(m)

                # minimal pre-pair front: only what pair (2,0) chunks
                # 0..7 need (kh2 + qB over the first x^T halves).
                # Everything else - kh0/kh1, b2/b3 projections, all V
                # chunks - rides inside pair (2,0), overlapping the
                # second x^T halves still streaming in. Block-0
                # output projection rides inside the block-1 pairs,
                # so only 8 proj chunks and the last flush remain in
                # the serial tail.
                for b in (0, 1):
                    g_group(b, 1)
                for b in (0, 1):
                    g_group(b, 2)
                ext0 = {0: [vv(0), gg(0, 0)], 1: [vv(1), gg(1, 0)],
                        2: [vv(2)], 3: [vv(3)],
                        4: [vv(4), gg(2, 1)], 5: [vv(5), gg(3, 1)],
                        6: [vv(6), gg(2, 0)], 7: [vv(7), gg(3, 0)],
                        8: [vv(8), gg(2, 2)], 9: [vv(9), gg(3, 2)]}
                for m in range(10, MC):
                    ext0[m] = [vv(m)]
                attn_pair(2, 0, ext0)
                attn_pair(0, 0)
                attn_pair(1, 0)
                # block-0 proj spread thinly over all block-1 pairs
                # ((2,1) only from chunk 8, after (1,0)'s normalize)
                attn_pair(2, 1, {8: [pj(0)], 11: [pj(1)]})
                attn_pair(0, 1, {4: [pj(2)], 8: [pj(3)], 12: [pj(4)]})
                attn_pair(1, 1, {4: [pj(5)], 8: [pj(6)], 12: [pj(7)]})
                while pend:
                    av_flush()
                # tail: run the ah2[1] halves of the first three
                # proj chunks under the last flush's normalize chain,
                # then pipeline phase1/phase2
                for m in range(8, 11):
                    proj1(m)
                for m in range(8, MC):
                    proj2(m)
                    if m + 3 < MC:
                        proj1(m + 3)

    nc.compile()
    return nc


def kernel(x, w_qkv, b_qkv, w_proj, b_proj):
    import ml_dtypes

    from concourse.bass_utils import run_bass_kernel_spmd

    global LAST_RESULTS
    if "nc" not in _cache:
        _cache["nc"] = _build()
    nc = _cache["nc"]

    bf16 = ml_dtypes.bfloat16
    x = np.asarray(x, dtype=np.float32)
    w_qkv = np.asarray(w_qkv, dtype=np.float32)
    b_qkv = np.asarray(b_qkv, dtype=np.float32)
    w_proj = np.asarray(w_proj, dtype=np.float32)
    b_proj = np.asarray(b_proj, dtype=np.float32)

    in_maps = []
    for c in range(NCORES):
        b = c // 4
        h0 = HPC * (c % 4)
        qs = slice(64 * h0, 64 * (h0 + HPC))
        ks = slice(C + 64 * h0, C + 64 * (h0 + HPC))
        vs = slice(2 * C + 64 * h0, 2 * C + 64 * (h0 + HPC))
        wkq = np.concatenate([w_qkv[:, ks], w_qkv[:, qs]], axis=1)
        wp_pad = np.zeros((2 * 128, C), dtype=np.float32)
        wp_pad[0:128] = w_proj[64 * h0:64 * (h0 + 2), :]
        wp_pad[128:192] = w_proj[64 * (h0 + 2):64 * (h0 + 3), :]
        in_maps.append({
            "xt": np.ascontiguousarray(x[b].T).astype(bf16),
            "wkq": np.ascontiguousarray(wkq).astype(bf16),
            "wv": np.ascontiguousarray(w_qkv[:, vs]).astype(bf16),
            "bq": np.ascontiguousarray(
                (b_qkv[qs] * 0.125).reshape(J, 1)).astype(np.float32),
            "wp": wp_pad.astype(bf16),
        })

    res = run_bass_kernel_spmd(nc, in_maps, core_ids=list(range(NCORES)))
    LAST_RESULTS = res

    out = np.zeros((B, N, C), dtype=np.float32)
    for c in range(NCORES):
        out[c // 4] += np.asarray(res.results[c]["out"], dtype=np.float32)
    out += b_proj + b_qkv[2 * C:] @ w_proj
    return out



# revision 61
# speedup vs baseline: 1.0091x; 1.0091x over previous
"""Multi-head attention block on 8 TRN2 NeuronCores.

Problem: x[2,2048,768] -> qkv proj -> 12-head attention -> out proj.
Sharding: 24 (batch, head) pairs across 8 cores; core c handles batch
c//4 and heads 3*(c%4)..3*(c%4)+2. Each core computes its heads'
Q,K,V, attention, and a partial output projection; the host sums the
four per-batch partials and adds the bias terms.

Design notes (v8, ~169us vs v6's ~176us):
  - Exp split across engines: odd key chunks compute exp on the DVE
    via a Schraudolph bit trick (one tensor_scalar:
    int16(s*184.665 + 16249) bitcast to bf16; the ~3% elementwise
    error mostly cancels in softmax, rel err 1.07e-2 vs the 2e-2
    gate). This halves the ACT exp stream, making the attention
    inner loop PE-bound (~98% TensorE occupancy) instead of
    ACT-bound. Pair-end raw copies move to ACT for the same reason.
  - One PSUM pool for the whole kernel: s tiles (3 bufs, 6 banks) +
    two AV accumulators (2 banks). Front work (QKV projections, V
    chunks) and the output projection borrow s slots, so they can
    interleave anywhere in the attention stream.
  - Minimal pre-pair front: only kh2/qB over the first x^T halves
    runs before attention; kh0/kh1, the b2/b3 projections and all V
    chunks ride inside pair (2,0) while the second x^T halves
    stream in (separate first/second-half x^T tiles keep the
    dependencies decoupled). Attention starts ~33us in (was ~47us).
  - Pair order (2,0),(0,0),(1,0),(2,1),(0,1),(1,1); the block-0
    output projection rides inside the block-1 pairs, and the tail
    runs the ah2[1] halves of the first three remaining proj chunks
    under the last flush's normalize chain (proj accumulates head 2
    first for this reason). Output DMA alternates sync/scalar
    queues so the final drain is not serialized.
  - All matmul operands bf16; output bf16; host sums partials in
    f32. K/Q fused weight stream, packed Q tiles, zero-carrying K
    tiles, ones-column V tiles for the softmax sums, DVE-memset
    constants and PE warmup matmuls carried over from v6.
  - Things measured NOT to help on this hardware: tile_position row
    tiling for the 64-deep QK contraction (no MM concurrency, and
    half-array matmuls re-throttle the HAM clock gate to 1.2GHz),
    1024-column moving matmuls (ISA caps at 512), xt DMA on the
    gpsimd queue, gpsimd tensor ops beyond partition_broadcast
    (multi-us library reloads), fp8 DoubleRow AV (needs fp8 exp
    output, which blows the error budget).
"""

import os
import sys

for _p in ("/opt/trn_rl_repo", "/opt/pypackages"):
    if _p not in sys.path:
        sys.path.append(_p)

import numpy as np

B, N, C = 2, 2048, 768
H, D = 12, 64
HPC = 3                    # heads per core
J = HPC * D                # 192 per-core head-dim rows
NCORES = 8
NBLK = 1024                # query-block width (one exp per [128, NBLK])
NB = N // NBLK             # 2
MC = N // 128              # 16 key chunks
KC = C // 128              # 6 contraction chunks for projections
NWARM = 12

SCH_A = 184.6649652337873   # 2^7 / ln 2
SCH_B = 16249.0             # exponent-bias offset, tuned for softmax

_cache = {}
LAST_RESULTS = None


def _build():
    import concourse.mybir as mybir
    import concourse.tile as tile
    from concourse import bacc

    f32 = mybir.dt.float32
    bf16 = mybir.dt.bfloat16
    i16 = mybir.dt.int16
    Exp = mybir.ActivationFunctionType.Exp
    Copy = mybir.ActivationFunctionType.Copy
    mult = mybir.AluOpType.mult
    add = mybir.AluOpType.add

    nc = bacc.Bacc("TRN2", target_bir_lowering=False, debug=False,
                   num_devices=NCORES)

    xt_d = nc.declare_dram_parameter("xt", [C, N], bf16, isOutput=False)
    # fused [K_h0|K_h1|K_h2|Q_h0|Q_h1|Q_h2] weight columns
    wkq_d = nc.declare_dram_parameter("wkq", [C, 2 * J], bf16,
                                      isOutput=False)
    wv_d = nc.declare_dram_parameter("wv", [C, J], bf16, isOutput=False)
    bq_d = nc.declare_dram_parameter("bq", [J, 1], f32, isOutput=False)
    # padded proj weights: rows 0:128 = heads 0,1; 128:192 = head 2;
    # 192:256 = zero (annihilates ah2[1]'s junk bottom half)
    wp_d = nc.declare_dram_parameter("wp", [2 * 128, C], bf16,
                                     isOutput=False)
    out_d = nc.declare_dram_parameter("out", [N, C], bf16, isOutput=True)

    with tile.TileContext(nc) as tc:
        with (
            tc.tile_pool(name="persist", bufs=1) as pp,
            tc.tile_pool(name="osb", bufs=4) as posb,
            tc.tile_pool(name="etile", bufs=6) as pe,
            tc.tile_pool(name="bcsb", bufs=2) as pbc,
        ):
            warm_t = pp.tile([128, 512], bf16, tag="warm_t", name="warm_t")
            wkq = [pp.tile([128, 2 * J], bf16, tag=f"wkq{k}",
                           name=f"wkq{k}") for k in range(KC)]
            # x^T halves as separate tiles so first-half
            # consumers never wait on the second-half DMA
            xtf = [pp.tile([128, 1024], bf16, tag=f"xtf{k}",
                           name=f"xtf{k}") for k in range(KC)]
            xts = [pp.tile([128, 1024], bf16, tag=f"xts{k}",
                           name=f"xts{k}") for k in range(KC)]
            wv = [pp.tile([128, J], bf16, tag=f"wv{k}", name=f"wv{k}")
                  for k in range(KC)]
            bqt = [pp.tile([64, 1], f32, tag=f"bq{h}", name=f"bq{h}")
                   for h in range(HPC)]
            wp = [pp.tile([128, C], bf16, tag=f"wp{t}", name=f"wp{t}")
                  for t in range(2)]
            # K^T per head, zero rows alternating so the packed Q tiles
            # need none: kh0 data 0:64, kh1 data 64:128, kh2 data 0:64
            kh = [pp.tile([128, N], bf16, tag=f"kh{h}", name=f"kh{h}")
                  for h in range(HPC)]
            # Q^T packed: qA = (q0 top, q1 bottom); qB = (q2 top, junk)
            qA = pp.tile([128, N], bf16, tag="qA", name="qA")
            qB = pp.tile([128, N], bf16, tag="qB", name="qB")
            # V with a ones column per head: [128, 3*65]
            vx = [pp.tile([128, HPC * 65], bf16, tag=f"vx{m}",
                          name=f"vx{m}") for m in range(MC)]
            sums = [pp.tile([1, N], f32, tag=f"sums{h}", name=f"sums{h}")
                    for h in range(HPC)]
            # unnormalized attention outputs (release AV PSUM fast),
            # base partition 0 so tensor_mul's SBUF inputs stay aligned
            raw = [pp.tile([64, N], bf16, tag=f"raw{h}", name=f"raw{h}")
                   for h in range(HPC)]
            ah2 = [pp.tile([128, N], bf16, tag=f"ah2{t}", name=f"ah2{t}")
                   for t in range(2)]

            # ---- constants via DVE memset (no DMA traffic; gpsimd
            # memset on partition-offset APs crashed the device) ----
            nc.vector.memset(warm_t[:], 1.0)
            nc.vector.memset(kh[0][64:128, :], 0.0)
            nc.vector.memset(kh[1][0:64, :], 0.0)
            nc.vector.memset(kh[2][64:128, :], 0.0)
            nc.vector.memset(qB[64:128, :], 0.0)
            nc.vector.memset(ah2[1][64:128, :], 0.0)
            for m in range(MC):
                on = vx[m].rearrange("p (h e) -> p h e", e=65)[:, :, 64:65]
                nc.vector.memset(on, 1.0)

            # ---- input DMA, ordered for earliest compute start ----
            # sync queue: x^T half-row tiles only (the long pole; each
            # extra trigger ahead of an xt tile delays it ~0.6us), first
            # halves feed blocks 0/1, then second halves
            for k in range(KC):
                nc.sync.dma_start(xtf[k][:],
                                  xt_d[128 * k:128 * (k + 1), 0:1024])
            for k in range(KC):
                nc.sync.dma_start(xts[k][:],
                                  xt_d[128 * k:128 * (k + 1), 1024:2048])
            # scalar queue (ACT idle early): fused KQ weights first
            # (k-th tile lands just ahead of the k-th accumulation
            # step; the tiny bias tiles would delay wkq[0] ~0.6us per
            # trigger), then q-bias, then V weights
            for k in range(KC):
                nc.scalar.dma_start(wkq[k][:], wkq_d[128 * k:128 * (k + 1), :])
            for h in range(HPC):
                nc.scalar.dma_start(bqt[h][:], bq_d[64 * h:64 * (h + 1), :])
            for k in range(KC):
                nc.scalar.dma_start(wv[k][:], wv_d[128 * k:128 * (k + 1), :])

            # wp is only needed by the projection tail; queue it after
            # the x^T burst
            for t in range(2):
                nc.sync.dma_start(wp[t][:], wp_d[128 * t:128 * (t + 1), :])

            with tc.tile_pool(name="ps2", bufs=1, space="PSUM") as ps2:
                # all front work (warmup, QKV projections, V chunks)
                # borrows "s" PSUM slots, so half of it can interleave
                # into the first attention pair while the second x^T
                # halves are still streaming in
                for i in range(NWARM):
                    ps = ps2.tile([128, NBLK], f32, tag="s", bufs=3,
                                  name=f"warm{i}")
                    nc.tensor.matmul(ps[:, 0:512], warm_t[:, 0:128],
                                     warm_t[:])

                def g_group(b, g):
                    # g0: K_h0,K_h1 / g1: K_h2,Q_h0 / g2: Q_h1,Q_h2
                    nsl = slice(512 * b, 512 * (b + 1))
                    xh = xtf if b < 2 else xts
                    hsl = slice(512 * (b % 2), 512 * (b % 2 + 1))
                    pf = ps2.tile([128, NBLK], f32, tag="s", bufs=3,
                                  name="ps_qk")
                    ps = pf[:, 0:512]
                    for k in range(KC):
                        nc.tensor.matmul(
                            ps, wkq[k][:, 128 * g:128 * (g + 1)],
                            xh[k][:, hsl],
                            start=(k == 0), stop=(k == KC - 1))
                    if g == 0:
                        nc.vector.tensor_copy(kh[0][0:64, nsl], ps[0:64, :])
                        nc.vector.tensor_copy(kh[1][64:128, nsl],
                                              ps[64:128, :])
                    elif g == 1:
                        nc.vector.tensor_copy(kh[2][0:64, nsl], ps[0:64, :])
                        nc.vector.tensor_scalar(
                            qA[0:64, nsl], ps[64:128, :], 0.125,
                            bqt[0][:], mult, add)
                    else:
                        nc.vector.tensor_scalar(
                            qA[64:128, nsl], ps[0:64, :], 0.125,
                            bqt[1][:], mult, add)
                        nc.vector.tensor_scalar(
                            qB[0:64, nsl], ps[64:128, :], 0.125,
                            bqt[2][:], mult, add)

                def v_chunk(m):
                    xh = xtf if m < 8 else xts
                    msl = slice(128 * (m % 8), 128 * (m % 8 + 1))
                    pf = ps2.tile([128, NBLK], f32, tag="s", bufs=3,
                                  name="ps_v")
                    ps = pf[:, 0:512]
                    for k in range(KC):
                        nc.tensor.matmul(ps[:, 0:J], xh[k][:, msl], wv[k][:],
                                         start=(k == 0), stop=(k == KC - 1))
                    vdst = vx[m].rearrange("p (h e) -> p h e",
                                           e=65)[:, :, 0:64]
                    nc.vector.tensor_copy(
                        vdst, ps[:, 0:J].rearrange("p (h e) -> p h e", e=64))

                pend = []

                def av_flush():
                    avh, h, nb, m, e = pend.pop(0)
                    vsl = slice(65 * h, 65 * (h + 1))
                    for i in range(2):
                        nc.tensor.matmul(
                            avh[i][:], vx[m][:, vsl],
                            e[:, 512 * i:512 * (i + 1)],
                            start=(m == 0), stop=(m == MC - 1))
                    if m != MC - 1:
                        return
                    adst, r0 = ((ah2[0], 0) if h == 0 else
                                (ah2[0], 64) if h == 1 else
                                (ah2[1], 0))
                    # raw copies first — they alone gate the next pair's
                    # AV PSUM writes; on ACT so the DVE exp stream is
                    # not disturbed. Sums rows and the broadcast/
                    # reciprocal/multiply chains follow off the critical
                    # path
                    for i in range(2):
                        hf = slice(NBLK * nb + 512 * i,
                                   NBLK * nb + 512 * (i + 1))
                        nc.scalar.activation(raw[h][:, hf],
                                             avh[i][0:64, :], Copy)
                    for i in range(2):
                        hf = slice(NBLK * nb + 512 * i,
                                   NBLK * nb + 512 * (i + 1))
                        nc.vector.tensor_copy(sums[h][:, hf],
                                              avh[i][64:65, :])
                    for i in range(2):
                        hf = slice(NBLK * nb + 512 * i,
                                   NBLK * nb + 512 * (i + 1))
                        bcs = pbc.tile([64, 512], f32, tag="bcs",
                                       name="bcs")
                        nc.gpsimd.partition_broadcast(bcs[:],
                                                      sums[h][:, hf])
                        rec = pbc.tile([64, 512], f32, tag="rec",
                                       name="rec")
                        nc.vector.reciprocal_approx_fast(rec[:], bcs[:])
                        nc.vector.tensor_mul(adst[r0:r0 + 64, hf],
                                             raw[h][:, hf], rec[:])

                def attn_pair(h, nb, extra=None):
                    # extra: dict chunk -> [thunks] fired after that
                    # chunk (interleaved projection / late front work)
                    qt = qA if h < 2 else qB
                    avh = [ps2.tile([65, 512], f32, tag=f"av{i}", bufs=1,
                                    name=f"ps_av{i}") for i in range(2)]
                    for m in range(MC):
                        msl = slice(128 * m, 128 * (m + 1))
                        s = ps2.tile([128, NBLK], f32, tag="s", bufs=3,
                                     name="ps_s")
                        for i in range(2):
                            q0 = NBLK * nb + 512 * i
                            nc.tensor.matmul(
                                s[:, 512 * i:512 * (i + 1)],
                                kh[h][:, msl], qt[:, q0:q0 + 512])
                        e = pe.tile([128, NBLK], bf16, tag="e", name="e")
                        if m % 2 == 1:
                            # odd chunks: Schraudolph exp on DVE
                            # (int16 bit trick, ~3% elementwise error
                            # that mostly cancels in softmax) to keep
                            # the ACT engine off the critical path
                            nc.vector.tensor_scalar(
                                e.bitcast(i16)[:], s[:],
                                SCH_A, SCH_B, mult, add)
                        else:
                            nc.scalar.activation(e[:], s[:], Exp)
                        pend.append((avh, h, nb, m, e))
                        if len(pend) > 2:
                            av_flush()
                        if extra and m in extra:
                            for th in extra[m]:
                                th()

                def proj(m):
                    # output projection chunk, borrowing an "s" PSUM
                    # slot so it can interleave into attention pairs;
                    # PSUM evacuation split across ACT and DVE
                    msl = slice(128 * m, 128 * (m + 1))
                    proj1(m)
                    proj2(m)

                pjt = {}

                def proj1(m):
                    # ah2[1] (head 2) part first: it is flushed two
                    # pairs before ah2[0], so these matmuls can run
                    # while the last pair's normalize chain completes
                    msl = slice(128 * m, 128 * (m + 1))
                    t = ps2.tile([128, NBLK], f32, tag="s", bufs=3,
                                 name="ps_pj")
                    pjt[m] = t
                    nc.tensor.matmul(t[:, 0:512], ah2[1][:, msl],
                                     wp[1][:, 0:512],
                                     start=True, stop=False)
                    nc.tensor.matmul(t[:, 512:768], ah2[1][:, msl],
                                     wp[1][:, 512:768],
                                     start=True, stop=False)

                def proj2(m):
                    msl = slice(128 * m, 128 * (m + 1))
                    t = pjt.pop(m)
                    nc.tensor.matmul(t[:, 0:512], ah2[0][:, msl],
                                     wp[0][:, 0:512],
                                     start=False, stop=True)
                    nc.tensor.matmul(t[:, 512:768], ah2[0][:, msl],
                                     wp[0][:, 512:768],
                                     start=False, stop=True)
                    o3 = posb.tile([128, C], bf16, tag="o3", name="o3")
                    nc.vector.tensor_copy(o3[:, 0:512], t[:, 0:512])
                    nc.scalar.activation(o3[:, 512:768], t[:, 512:768],
                                         Copy)
                    # rotate output queues so the final DMA drain
                    # is not serialized on one ring
                    oq = [nc.sync, nc.scalar, nc.gpsimd][m % 3]
                    oq.dma_start(out_d[msl, :], o3[:])

                def gg(b, g):
                    return lambda: g_group(b, g)

                def vv(m):
                    return lambda: v_chunk(m)

                def pj(m):
                    return lambda: proj(m)

                # minimal pre-pair front: only what pair (2,0) chunks
                # 0..7 need (kh2 + qB over the first x^T halves).
                # Everything else - kh0/kh1, b2/b3 projections, all V
                # chunks - rides inside pair (2,0), overlapping the
                # second x^T halves still streaming in. Block-0
                # output projection rides inside the block-1 pairs,
                # so only 8 proj chunks and the last flush remain in
                # the serial tail.
                for b in (0, 1):
                    g_group(b, 1)
                for b in (0, 1):
                    g_group(b, 2)
                ext0 = {0: [vv(0), gg(0, 0)], 1: [vv(1), gg(1, 0)],
                        2: [vv(2)], 3: [vv(3)],
                        4: [vv(4), gg(2, 1)], 5: [vv(5), gg(3, 1)],
                        6: [vv(6), gg(2, 0)], 7: [vv(7), gg(3, 0)],
                        8: [vv(8), gg(2, 2)], 9: [vv(9), gg(3, 2)]}
                for m in range(10, MC):
                    ext0[m] = [vv(m)]
                attn_pair(2, 0, ext0)
                attn_pair(0, 0)
                attn_pair(1, 0)
                # block-0 proj spread thinly over all block-1 pairs
                # ((2,1) only from chunk 8, after (1,0)'s normalize)
                attn_pair(2, 1, {8: [pj(0)], 11: [pj(1)]})
                attn_pair(0, 1, {4: [pj(2)], 8: [pj(3)], 12: [pj(4)]})
                attn_pair(1, 1, {4: [pj(5)], 8: [pj(6)], 12: [pj(7)]})
                while pend:
                    av_flush()
                # tail: run the ah2[1] halves of the first three
                # proj chunks under the last flush's normalize chain,
                # then pipeline phase1/phase2
                for m in range(8, 11):
                    proj1(m)
                for m in range(8, MC):
                    proj2(m)
                    if m + 3 < MC:
                        proj1(m + 3)

    nc.compile()
    return nc


def kernel(x, w_qkv, b_qkv, w_proj, b_proj):
    import ml_dtypes

    from concourse.bass_utils import run_bass_kernel_spmd

    global LAST_RESULTS
    if "nc" not in _cache:
        _cache["nc"] = _build()
    nc = _cache["nc"]

    bf16 = ml_dtypes.bfloat16
    x = np.asarray(x, dtype=np.float32)
    w_qkv = np.asarray(w_qkv, dtype=np.float32)
    b_qkv = np.asarray(b_qkv, dtype=np.float32)
    w_proj = np.asarray(w_proj, dtype=np.float32)
    b_proj = np.asarray(b_proj, dtype=np.float32)

    in_maps = []
    for c in range(NCORES):
        b = c // 4
        h0 = HPC * (c % 4)
        qs = slice(64 * h0, 64 * (h0 + HPC))
        ks = slice(C + 64 * h0, C + 64 * (h0 + HPC))
        vs = slice(2 * C + 64 * h0, 2 * C + 64 * (h0 + HPC))
        wkq = np.concatenate([w_qkv[:, ks], w_qkv[:, qs]], axis=1)
        wp_pad = np.zeros((2 * 128, C), dtype=np.float32)
        wp_pad[0:128] = w_proj[64 * h0:64 * (h0 + 2), :]
        wp_pad[128:192] = w_proj[64 * (h0 + 2):64 * (h0 + 3), :]
        in_maps.append({
            "xt": np.ascontiguousarray(x[b].T).astype(bf16),
            "wkq": np.ascontiguousarray(wkq).astype(bf16),
            "wv": np.ascontiguousarray(w_qkv[:, vs]).astype(bf16),
            "bq": np.ascontiguousarray(
                (b_qkv[qs] * 0.125).reshape(J, 1)).astype(np.float32),
            "wp": wp_pad.astype(bf16),
        })

    res = run_bass_kernel_spmd(nc, in_maps, core_ids=list(range(NCORES)))
    LAST_RESULTS = res

    out = np.zeros((B, N, C), dtype=np.float32)
    for c in range(NCORES):
        out[c // 4] += np.asarray(res.results[c]["out"], dtype=np.float32)
    out += b_proj + b_qkv[2 * C:] @ w_proj
    return out



# revision 62
# speedup vs baseline: 1.0235x; 1.0142x over previous
"""Multi-head attention block on 8 TRN2 NeuronCores.

Problem: x[2,2048,768] -> qkv proj -> 12-head attention -> out proj.
Sharding: 24 (batch, head) pairs across 8 cores; core c handles batch
c//4 and heads 3*(c%4)..3*(c%4)+2. Each core computes its heads'
Q,K,V, attention, and a partial output projection; the host sums the
four per-batch partials and adds the bias terms.

Design notes (v8, ~169us vs v6's ~176us):
  - Exp split across engines: odd key chunks compute exp on the DVE
    via a Schraudolph bit trick (one tensor_scalar:
    int16(s*184.665 + 16249) bitcast to bf16; the ~3% elementwise
    error mostly cancels in softmax, rel err 1.07e-2 vs the 2e-2
    gate). This halves the ACT exp stream, making the attention
    inner loop PE-bound (~98% TensorE occupancy) instead of
    ACT-bound. Pair-end raw copies move to ACT for the same reason.
  - One PSUM pool for the whole kernel: s tiles (3 bufs, 6 banks) +
    two AV accumulators (2 banks). Front work (QKV projections, V
    chunks) and the output projection borrow s slots, so they can
    interleave anywhere in the attention stream.
  - Minimal pre-pair front: only kh2/qB over the first x^T halves
    runs before attention; kh0/kh1, the b2/b3 projections and all V
    chunks ride inside pair (2,0) while the second x^T halves
    stream in (separate first/second-half x^T tiles keep the
    dependencies decoupled). Attention starts ~33us in (was ~47us).
  - Pair order (2,0),(0,0),(1,0),(2,1),(0,1),(1,1); the block-0
    output projection rides inside the block-1 pairs, and the tail
    runs the ah2[1] halves of the first three remaining proj chunks
    under the last flush's normalize chain (proj accumulates head 2
    first for this reason). Output DMA alternates sync/scalar
    queues so the final drain is not serialized.
  - All matmul operands bf16; output bf16; host sums partials in
    f32. K/Q fused weight stream, packed Q tiles, zero-carrying K
    tiles, ones-column V tiles for the softmax sums, DVE-memset
    constants and PE warmup matmuls carried over from v6.
  - Things measured NOT to help on this hardware: tile_position row
    tiling for the 64-deep QK contraction (no MM concurrency, and
    half-array matmuls re-throttle the HAM clock gate to 1.2GHz),
    1024-column moving matmuls (ISA caps at 512), xt DMA on the
    gpsimd queue, gpsimd tensor ops beyond partition_broadcast
    (multi-us library reloads), fp8 DoubleRow AV (needs fp8 exp
    output, which blows the error budget).
"""

import os
import sys

for _p in ("/opt/trn_rl_repo", "/opt/pypackages"):
    if _p not in sys.path:
        sys.path.append(_p)

import numpy as np

B, N, C = 2, 2048, 768
H, D = 12, 64
HPC = 3                    # heads per core
J = HPC * D                # 192 per-core head-dim rows
NCORES = 8
NBLK = 1024                # query-block width (one exp per [128, NBLK])
NB = N // NBLK             # 2
MC = N // 128              # 16 key chunks
KC = C // 128              # 6 contraction chunks for projections
NWARM = 12

SCH_A = 184.6649652337873   # 2^7 / ln 2
SCH_B = 16249.0             # exponent-bias offset, tuned for softmax

_cache = {}
LAST_RESULTS = None


def _build():
    import concourse.mybir as mybir
    import concourse.tile as tile
    from concourse import bacc

    f32 = mybir.dt.float32
    bf16 = mybir.dt.bfloat16
    i16 = mybir.dt.int16
    Exp = mybir.ActivationFunctionType.Exp
    Copy = mybir.ActivationFunctionType.Copy
    mult = mybir.AluOpType.mult
    add = mybir.AluOpType.add

    nc = bacc.Bacc("TRN2", target_bir_lowering=False, debug=False,
                   num_devices=NCORES)

    xt_d = nc.declare_dram_parameter("xt", [C, N], bf16, isOutput=False)
    # fused [K_h0|K_h1|K_h2|Q_h0|Q_h1|Q_h2] weight columns
    wkq_d = nc.declare_dram_parameter("wkq", [C, 2 * J], bf16,
                                      isOutput=False)
    wv_d = nc.declare_dram_parameter("wv", [C, J], bf16, isOutput=False)
    bq_d = nc.declare_dram_parameter("bq", [J, 1], f32, isOutput=False)
    # padded proj weights: rows 0:128 = heads 0,1; 128:192 = head 2;
    # 192:256 = zero (annihilates ah2[1]'s junk bottom half)
    wp_d = nc.declare_dram_parameter("wp", [2 * 128, C], bf16,
                                     isOutput=False)
    out_d = nc.declare_dram_parameter("out", [N, C], bf16, isOutput=True)

    with tile.TileContext(nc) as tc:
        with (
            tc.tile_pool(name="persist", bufs=1) as pp,
            tc.tile_pool(name="osb", bufs=4) as posb,
            tc.tile_pool(name="etile", bufs=6) as pe,
            tc.tile_pool(name="bcsb", bufs=2) as pbc,
        ):
            warm_t = pp.tile([128, 512], bf16, tag="warm_t", name="warm_t")
            wkq = [pp.tile([128, 2 * J], bf16, tag=f"wkq{k}",
                           name=f"wkq{k}") for k in range(KC)]
            # x^T halves as separate tiles so first-half
            # consumers never wait on the second-half DMA
            xtf = [pp.tile([128, 1024], bf16, tag=f"xtf{k}",
                           name=f"xtf{k}") for k in range(KC)]
            xts = [pp.tile([128, 1024], bf16, tag=f"xts{k}",
                           name=f"xts{k}") for k in range(KC)]
            wv = [pp.tile([128, J], bf16, tag=f"wv{k}", name=f"wv{k}")
                  for k in range(KC)]
            bqt = [pp.tile([64, 1], f32, tag=f"bq{h}", name=f"bq{h}")
                   for h in range(HPC)]
            wp = [pp.tile([128, C], bf16, tag=f"wp{t}", name=f"wp{t}")
                  for t in range(2)]
            # K^T per head, zero rows alternating so the packed Q tiles
            # need none: kh0 data 0:64, kh1 data 64:128, kh2 data 0:64
            kh = [pp.tile([128, N], bf16, tag=f"kh{h}", name=f"kh{h}")
                  for h in range(HPC)]
            # Q^T packed: qA = (q0 top, q1 bottom); qB = (q2 top, junk)
            qA = pp.tile([128, N], bf16, tag="qA", name="qA")
            qB = pp.tile([128, N], bf16, tag="qB", name="qB")
            # V with a ones column per head: [128, 3*65]
            vx = [pp.tile([128, HPC * 65], bf16, tag=f"vx{m}",
                          name=f"vx{m}") for m in range(MC)]
            sums = [pp.tile([1, N], f32, tag=f"sums{h}", name=f"sums{h}")
                    for h in range(HPC)]
            # unnormalized attention outputs (release AV PSUM fast),
            # base partition 0 so tensor_mul's SBUF inputs stay aligned
            raw = [pp.tile([64, N], bf16, tag=f"raw{h}", name=f"raw{h}")
                   for h in range(HPC)]
            ah2 = [pp.tile([128, N], bf16, tag=f"ah2{t}", name=f"ah2{t}")
                   for t in range(2)]

            # ---- constants via DVE memset (no DMA traffic; gpsimd
            # memset on partition-offset APs crashed the device) ----
            nc.vector.memset(warm_t[:], 1.0)
            nc.vector.memset(kh[0][64:128, :], 0.0)
            nc.vector.memset(kh[1][0:64, :], 0.0)
            nc.vector.memset(kh[2][64:128, :], 0.0)
            nc.vector.memset(qB[64:128, :], 0.0)
            nc.vector.memset(ah2[1][64:128, :], 0.0)
            for m in range(MC):
                on = vx[m].rearrange("p (h e) -> p h e", e=65)[:, :, 64:65]
                nc.vector.memset(on, 1.0)

            # ---- input DMA, ordered for earliest compute start ----
            # sync queue: x^T half-row tiles only (the long pole; each
            # extra trigger ahead of an xt tile delays it ~0.6us), first
            # halves feed blocks 0/1, then second halves
            for k in range(KC):
                nc.sync.dma_start(xtf[k][:],
                                  xt_d[128 * k:128 * (k + 1), 0:1024])
            for k in range(KC):
                nc.sync.dma_start(xts[k][:],
                                  xt_d[128 * k:128 * (k + 1), 1024:2048])
            # scalar queue (ACT idle early): fused KQ weights first
            # (k-th tile lands just ahead of the k-th accumulation
            # step; the tiny bias tiles would delay wkq[0] ~0.6us per
            # trigger), then q-bias, then V weights
            for k in range(KC):
                nc.scalar.dma_start(wkq[k][:], wkq_d[128 * k:128 * (k + 1), :])
            for h in range(HPC):
                nc.scalar.dma_start(bqt[h][:], bq_d[64 * h:64 * (h + 1), :])
            for k in range(KC):
                nc.scalar.dma_start(wv[k][:], wv_d[128 * k:128 * (k + 1), :])

            # wp is only needed by the projection tail; queue it after
            # the x^T burst
            for t in range(2):
                nc.sync.dma_start(wp[t][:], wp_d[128 * t:128 * (t + 1), :])

            with tc.tile_pool(name="ps2", bufs=1, space="PSUM") as ps2:
                # all front work (warmup, QKV projections, V chunks)
                # borrows "s" PSUM slots, so half of it can interleave
                # into the first attention pair while the second x^T
                # halves are still streaming in
                for i in range(NWARM):
                    ps = ps2.tile([128, NBLK], f32, tag="s", bufs=3,
                                  name=f"warm{i}")
                    nc.tensor.matmul(ps[:, 0:512], warm_t[:, 0:128],
                                     warm_t[:])

                def g_group(b, g):
                    # g0: K_h0,K_h1 / g1: K_h2,Q_h0 / g2: Q_h1,Q_h2
                    nsl = slice(512 * b, 512 * (b + 1))
                    xh = xtf if b < 2 else xts
                    hsl = slice(512 * (b % 2), 512 * (b % 2 + 1))
                    pf = ps2.tile([128, NBLK], f32, tag="s", bufs=3,
                                  name="ps_qk")
                    ps = pf[:, 0:512]
                    for k in range(KC):
                        nc.tensor.matmul(
                            ps, wkq[k][:, 128 * g:128 * (g + 1)],
                            xh[k][:, hsl],
                            start=(k == 0), stop=(k == KC - 1))
                    if g == 0:
                        nc.vector.tensor_copy(kh[0][0:64, nsl], ps[0:64, :])
                        nc.vector.tensor_copy(kh[1][64:128, nsl],
                                              ps[64:128, :])
                    elif g == 1:
                        nc.vector.tensor_copy(kh[2][0:64, nsl], ps[0:64, :])
                        nc.vector.tensor_scalar(
                            qA[0:64, nsl], ps[64:128, :], 0.125,
                            bqt[0][:], mult, add)
                    else:
                        nc.vector.tensor_scalar(
                            qA[64:128, nsl], ps[0:64, :], 0.125,
                            bqt[1][:], mult, add)
                        nc.vector.tensor_scalar(
                            qB[0:64, nsl], ps[64:128, :], 0.125,
                            bqt[2][:], mult, add)

                def v_chunk(m):
                    xh = xtf if m < 8 else xts
                    msl = slice(128 * (m % 8), 128 * (m % 8 + 1))
                    pf = ps2.tile([128, NBLK], f32, tag="s", bufs=3,
                                  name="ps_v")
                    ps = pf[:, 0:512]
                    for k in range(KC):
                        nc.tensor.matmul(ps[:, 0:J], xh[k][:, msl], wv[k][:],
                                         start=(k == 0), stop=(k == KC - 1))
                    vdst = vx[m].rearrange("p (h e) -> p h e",
                                           e=65)[:, :, 0:64]
                    nc.vector.tensor_copy(
                        vdst, ps[:, 0:J].rearrange("p (h e) -> p h e", e=64))

                pend = []

                def av_flush():
                    avh, h, nb, m, e = pend.pop(0)
                    vsl = slice(65 * h, 65 * (h + 1))
                    for i in range(2):
                        nc.tensor.matmul(
                            avh[i][:], vx[m][:, vsl],
                            e[:, 512 * i:512 * (i + 1)],
                            start=(m == 0), stop=(m == MC - 1))
                    if m != MC - 1:
                        return
                    adst, r0 = ((ah2[0], 0) if h == 0 else
                                (ah2[0], 64) if h == 1 else
                                (ah2[1], 0))
                    # raw copies first — they alone gate the next pair's
                    # AV PSUM writes; on ACT so the DVE exp stream is
                    # not disturbed. Sums rows and the broadcast/
                    # reciprocal/multiply chains follow off the critical
                    # path
                    for i in range(2):
                        hf = slice(NBLK * nb + 512 * i,
                                   NBLK * nb + 512 * (i + 1))
                        nc.scalar.activation(raw[h][:, hf],
                                             avh[i][0:64, :], Copy)
                    for i in range(2):
                        hf = slice(NBLK * nb + 512 * i,
                                   NBLK * nb + 512 * (i + 1))
                        nc.vector.tensor_copy(sums[h][:, hf],
                                              avh[i][64:65, :])
                    for i in range(2):
                        hf = slice(NBLK * nb + 512 * i,
                                   NBLK * nb + 512 * (i + 1))
                        bcs = pbc.tile([64, 512], f32, tag="bcs",
                                       name="bcs")
                        nc.gpsimd.partition_broadcast(bcs[:],
                                                      sums[h][:, hf])
                        rec = pbc.tile([64, 512], f32, tag="rec",
                                       name="rec")
                        nc.vector.reciprocal_approx_fast(rec[:], bcs[:])
                        nc.vector.tensor_mul(adst[r0:r0 + 64, hf],
                                             raw[h][:, hf], rec[:])

                def attn_pair(h, nb, extra=None):
                    # extra: dict chunk -> [thunks] fired after that
                    # chunk (interleaved projection / late front work)
                    qt = qA if h < 2 else qB
                    avh = [ps2.tile([65, 512], f32, tag=f"av{i}", bufs=1,
                                    name=f"ps_av{i}") for i in range(2)]
                    for m in range(MC):
                        msl = slice(128 * m, 128 * (m + 1))
                        s = ps2.tile([128, NBLK], f32, tag="s", bufs=3,
                                     name="ps_s")
                        for i in range(2):
                            q0 = NBLK * nb + 512 * i
                            nc.tensor.matmul(
                                s[:, 512 * i:512 * (i + 1)],
                                kh[h][:, msl], qt[:, q0:q0 + 512])
                        e = pe.tile([128, NBLK], bf16, tag="e", name="e")
                        if m % 2 == 1:
                            # odd chunks: Schraudolph exp on DVE
                            # (int16 bit trick, ~3% elementwise error
                            # that mostly cancels in softmax) to keep
                            # the ACT engine off the critical path
                            nc.vector.tensor_scalar(
                                e.bitcast(i16)[:], s[:],
                                SCH_A, SCH_B, mult, add)
                        else:
                            nc.scalar.activation(e[:], s[:], Exp)
                        pend.append((avh, h, nb, m, e))
                        if len(pend) > 3:
                            av_flush()
                        if extra and m in extra:
                            for th in extra[m]:
                                th()

                def proj(m):
                    # output projection chunk, borrowing an "s" PSUM
                    # slot so it can interleave into attention pairs;
                    # PSUM evacuation split across ACT and DVE
                    msl = slice(128 * m, 128 * (m + 1))
                    proj1(m)
                    proj2(m)

                pjt = {}

                def proj1(m):
                    # ah2[1] (head 2) part first: it is flushed two
                    # pairs before ah2[0], so these matmuls can run
                    # while the last pair's normalize chain completes
                    msl = slice(128 * m, 128 * (m + 1))
                    t = ps2.tile([128, NBLK], f32, tag="s", bufs=3,
                                 name="ps_pj")
                    pjt[m] = t
                    nc.tensor.matmul(t[:, 0:512], ah2[1][:, msl],
                                     wp[1][:, 0:512],
                                     start=True, stop=False)
                    nc.tensor.matmul(t[:, 512:768], ah2[1][:, msl],
                                     wp[1][:, 512:768],
                                     start=True, stop=False)

                def proj2(m):
                    msl = slice(128 * m, 128 * (m + 1))
                    t = pjt.pop(m)
                    nc.tensor.matmul(t[:, 0:512], ah2[0][:, msl],
                                     wp[0][:, 0:512],
                                     start=False, stop=True)
                    nc.tensor.matmul(t[:, 512:768], ah2[0][:, msl],
                                     wp[0][:, 512:768],
                                     start=False, stop=True)
                    o3 = posb.tile([128, C], bf16, tag="o3", name="o3")
                    nc.vector.tensor_copy(o3[:, 0:512], t[:, 0:512])
                    nc.scalar.activation(o3[:, 512:768], t[:, 512:768],
                                         Copy)
                    # rotate output queues so the final DMA drain
                    # is not serialized on one ring
                    oq = [nc.sync, nc.scalar, nc.gpsimd][m % 3]
                    oq.dma_start(out_d[msl, :], o3[:])

                def gg(b, g):
                    return lambda: g_group(b, g)

                def vv(m):
                    return lambda: v_chunk(m)

                def pj(m):
                    return lambda: proj(m)

                # minimal pre-pair front: only what pair (2,0) chunks
                # 0..7 need (kh2 + qB over the first x^T halves).
                # Everything else - kh0/kh1, b2/b3 projections, all V
                # chunks - rides inside pair (2,0), overlapping the
                # second x^T halves still streaming in. Block-0
                # output projection rides inside the block-1 pairs,
                # so only 8 proj chunks and the last flush remain in
                # the serial tail.
                for b in (0, 1):
                    g_group(b, 1)
                for b in (0, 1):
                    g_group(b, 2)
                ext0 = {0: [vv(0), gg(0, 0)], 1: [vv(1), gg(1, 0)],
                        2: [vv(2)], 3: [vv(3)],
                        4: [vv(4), gg(2, 1)], 5: [vv(5), gg(3, 1)],
                        6: [vv(6), gg(2, 0)], 7: [vv(7), gg(3, 0)],
                        8: [vv(8), gg(2, 2)], 9: [vv(9), gg(3, 2)]}
                for m in range(10, MC):
                    ext0[m] = [vv(m)]
                attn_pair(2, 0, ext0)
                attn_pair(0, 0)
                attn_pair(1, 0)
                # block-0 proj spread thinly over all block-1 pairs
                # ((2,1) only from chunk 8, after (1,0)'s normalize)
                attn_pair(2, 1, {8: [pj(0)], 11: [pj(1)]})
                attn_pair(0, 1, {4: [pj(2)], 8: [pj(3)], 12: [pj(4)]})
                attn_pair(1, 1, {4: [pj(5)], 8: [pj(6)], 12: [pj(7)]})
                while pend:
                    av_flush()
                # tail: run the ah2[1] halves of the first three
                # proj chunks under the last flush's normalize chain,
                # then pipeline phase1/phase2
                for m in range(8, 11):
                    proj1(m)
                for m in range(8, MC):
                    proj2(m)
                    if m + 3 < MC:
                        proj1(m + 3)

    nc.compile()
    return nc


def kernel(x, w_qkv, b_qkv, w_proj, b_proj):
    import ml_dtypes

    from concourse.bass_utils import run_bass_kernel_spmd

    global LAST_RESULTS
    if "nc" not in _cache:
        _cache["nc"] = _build()
    nc = _cache["nc"]

    bf16 = ml_dtypes.bfloat16
    x = np.asarray(x, dtype=np.float32)
    w_qkv = np.asarray(w_qkv, dtype=np.float32)
    b_qkv = np.asarray(b_qkv, dtype=np.float32)
    w_proj = np.asarray(w_proj, dtype=np.float32)
    b_proj = np.asarray(b_proj, dtype=np.float32)

    in_maps = []
    for c in range(NCORES):
        b = c // 4
        h0 = HPC * (c % 4)
        qs = slice(64 * h0, 64 * (h0 + HPC))
        ks = slice(C + 64 * h0, C + 64 * (h0 + HPC))
        vs = slice(2 * C + 64 * h0, 2 * C + 64 * (h0 + HPC))
        wkq = np.concatenate([w_qkv[:, ks], w_qkv[:, qs]], axis=1)
        wp_pad = np.zeros((2 * 128, C), dtype=np.float32)
        wp_pad[0:128] = w_proj[64 * h0:64 * (h0 + 2), :]
        wp_pad[128:192] = w_proj[64 * (h0 + 2):64 * (h0 + 3), :]
        in_maps.append({
            "xt": np.ascontiguousarray(x[b].T).astype(bf16),
            "wkq": np.ascontiguousarray(wkq).astype(bf16),
            "wv": np.ascontiguousarray(w_qkv[:, vs]).astype(bf16),
            "bq": np.ascontiguousarray(
                (b_qkv[qs] * 0.125).reshape(J, 1)).astype(np.float32),
            "wp": wp_pad.astype(bf16),
        })

    res = run_bass_kernel_spmd(nc, in_maps, core_ids=list(range(NCORES)))
    LAST_RESULTS = res

    out = np.zeros((B, N, C), dtype=np.float32)
    for c in range(NCORES):
        out[c // 4] += np.asarray(res.results[c]["out"], dtype=np.float32)
    out += b_proj + b_qkv[2 * C:] @ w_proj
    return out



# revision 63
# speedup vs baseline: 1.0319x; 1.0082x over previous
"""Multi-head attention block on 8 TRN2 NeuronCores.

Problem: x[2,2048,768] -> qkv proj -> 12-head attention -> out proj.
Sharding: 24 (batch, head) pairs across 8 cores; core c handles batch
c//4 and heads 3*(c%4)..3*(c%4)+2. Each core computes its heads'
Q,K,V, attention, and a partial output projection; the host sums the
four per-batch partials and adds the bias terms.

Design notes (v8, ~169us vs v6's ~176us):
  - Exp split across engines: odd key chunks compute exp on the DVE
    via a Schraudolph bit trick (one tensor_scalar:
    int16(s*184.665 + 16249) bitcast to bf16; the ~3% elementwise
    error mostly cancels in softmax, rel err 1.07e-2 vs the 2e-2
    gate). This halves the ACT exp stream, making the attention
    inner loop PE-bound (~98% TensorE occupancy) instead of
    ACT-bound. Pair-end raw copies move to ACT for the same reason.
  - One PSUM pool for the whole kernel: s tiles (3 bufs, 6 banks) +
    two AV accumulators (2 banks). Front work (QKV projections, V
    chunks) and the output projection borrow s slots, so they can
    interleave anywhere in the attention stream.
  - Minimal pre-pair front: only kh2/qB over the first x^T halves
    runs before attention; kh0/kh1, the b2/b3 projections and all V
    chunks ride inside pair (2,0) while the second x^T halves
    stream in (separate first/second-half x^T tiles keep the
    dependencies decoupled). Attention starts ~33us in (was ~47us).
  - Pair order (2,0),(0,0),(1,0),(2,1),(0,1),(1,1); the block-0
    output projection rides inside the block-1 pairs, and the tail
    runs the ah2[1] halves of the first three remaining proj chunks
    under the last flush's normalize chain (proj accumulates head 2
    first for this reason). Output DMA alternates sync/scalar
    queues so the final drain is not serialized.
  - All matmul operands bf16; output bf16; host sums partials in
    f32. K/Q fused weight stream, packed Q tiles, zero-carrying K
    tiles, ones-column V tiles for the softmax sums, DVE-memset
    constants and PE warmup matmuls carried over from v6.
  - Things measured NOT to help on this hardware: tile_position row
    tiling for the 64-deep QK contraction (no MM concurrency, and
    half-array matmuls re-throttle the HAM clock gate to 1.2GHz),
    1024-column moving matmuls (ISA caps at 512), xt DMA on the
    gpsimd queue, gpsimd tensor ops beyond partition_broadcast
    (multi-us library reloads), fp8 DoubleRow AV (needs fp8 exp
    output, which blows the error budget).
"""

import os
import sys

for _p in ("/opt/trn_rl_repo", "/opt/pypackages"):
    if _p not in sys.path:
        sys.path.append(_p)

import numpy as np

B, N, C = 2, 2048, 768
H, D = 12, 64
HPC = 3                    # heads per core
J = HPC * D                # 192 per-core head-dim rows
NCORES = 8
NBLK = 1024                # query-block width (one exp per [128, NBLK])
NB = N // NBLK             # 2
MC = N // 128              # 16 key chunks
KC = C // 128              # 6 contraction chunks for projections
NWARM = 12

SCH_A = 184.6649652337873   # 2^7 / ln 2
SCH_B = 16249.0             # exponent-bias offset, tuned for softmax

_cache = {}
LAST_RESULTS = None


def _build():
    import concourse.mybir as mybir
    import concourse.tile as tile
    from concourse import bacc

    f32 = mybir.dt.float32
    bf16 = mybir.dt.bfloat16
    i16 = mybir.dt.int16
    Exp = mybir.ActivationFunctionType.Exp
    Copy = mybir.ActivationFunctionType.Copy
    mult = mybir.AluOpType.mult
    add = mybir.AluOpType.add

    nc = bacc.Bacc("TRN2", target_bir_lowering=False, debug=False,
                   num_devices=NCORES)

    xt_d = nc.declare_dram_parameter("xt", [C, N], bf16, isOutput=False)
    # fused [K_h0|K_h1|K_h2|Q_h0|Q_h1|Q_h2] weight columns
    wkq_d = nc.declare_dram_parameter("wkq", [C, 2 * J], bf16,
                                      isOutput=False)
    wv_d = nc.declare_dram_parameter("wv", [C, J], bf16, isOutput=False)
    bq_d = nc.declare_dram_parameter("bq", [J, 1], f32, isOutput=False)
    # padded proj weights: rows 0:128 = heads 0,1; 128:192 = head 2;
    # 192:256 = zero (annihilates ah2[1]'s junk bottom half)
    wp_d = nc.declare_dram_parameter("wp", [2 * 128, C], bf16,
                                     isOutput=False)
    out_d = nc.declare_dram_parameter("out", [N, C], bf16, isOutput=True)

    with tile.TileContext(nc) as tc:
        with (
            tc.tile_pool(name="persist", bufs=1) as pp,
            tc.tile_pool(name="osb", bufs=4) as posb,
            tc.tile_pool(name="etile", bufs=8) as pe,
            tc.tile_pool(name="bcsb", bufs=2) as pbc,
        ):
            warm_t = pp.tile([128, 512], bf16, tag="warm_t", name="warm_t")
            wkq = [pp.tile([128, 2 * J], bf16, tag=f"wkq{k}",
                           name=f"wkq{k}") for k in range(KC)]
            # x^T halves as separate tiles so first-half
            # consumers never wait on the second-half DMA
            xtf = [pp.tile([128, 1024], bf16, tag=f"xtf{k}",
                           name=f"xtf{k}") for k in range(KC)]
            xts = [pp.tile([128, 1024], bf16, tag=f"xts{k}",
                           name=f"xts{k}") for k in range(KC)]
            wv = [pp.tile([128, J], bf16, tag=f"wv{k}", name=f"wv{k}")
                  for k in range(KC)]
            bqt = [pp.tile([64, 1], f32, tag=f"bq{h}", name=f"bq{h}")
                   for h in range(HPC)]
            wp = [pp.tile([128, C], bf16, tag=f"wp{t}", name=f"wp{t}")
                  for t in range(2)]
            # K^T per head, zero rows alternating so the packed Q tiles
            # need none: kh0 data 0:64, kh1 data 64:128, kh2 data 0:64
            kh = [pp.tile([128, N], bf16, tag=f"kh{h}", name=f"kh{h}")
                  for h in range(HPC)]
            # Q^T packed: qA = (q0 top, q1 bottom); qB = (q2 top, junk)
            qA = pp.tile([128, N], bf16, tag="qA", name="qA")
            qB = pp.tile([128, N], bf16, tag="qB", name="qB")
            # V with a ones column per head: [128, 3*65]
            vx = [pp.tile([128, HPC * 65], bf16, tag=f"vx{m}",
                          name=f"vx{m}") for m in range(MC)]
            sums = [pp.tile([1, N], f32, tag=f"sums{h}", name=f"sums{h}")
                    for h in range(HPC)]
            # unnormalized attention outputs (release AV PSUM fast),
            # base partition 0 so tensor_mul's SBUF inputs stay aligned
            raw = [pp.tile([64, N], bf16, tag=f"raw{h}", name=f"raw{h}")
                   for h in range(HPC)]
            ah2 = [pp.tile([128, N], bf16, tag=f"ah2{t}", name=f"ah2{t}")
                   for t in range(2)]

            # ---- constants via DVE memset (no DMA traffic; gpsimd
            # memset on partition-offset APs crashed the device) ----
            nc.vector.memset(warm_t[:], 1.0)
            nc.vector.memset(kh[0][64:128, :], 0.0)
            nc.vector.memset(kh[1][0:64, :], 0.0)
            nc.vector.memset(kh[2][64:128, :], 0.0)
            nc.vector.memset(qB[64:128, :], 0.0)
            nc.vector.memset(ah2[1][64:128, :], 0.0)
            for m in range(MC):
                on = vx[m].rearrange("p (h e) -> p h e", e=65)[:, :, 64:65]
                nc.vector.memset(on, 1.0)

            # ---- input DMA, ordered for earliest compute start ----
            # sync queue: x^T half-row tiles only (the long pole; each
            # extra trigger ahead of an xt tile delays it ~0.6us), first
            # halves feed blocks 0/1, then second halves
            for k in range(KC):
                nc.sync.dma_start(xtf[k][:],
                                  xt_d[128 * k:128 * (k + 1), 0:1024])
            for k in range(KC):
                nc.sync.dma_start(xts[k][:],
                                  xt_d[128 * k:128 * (k + 1), 1024:2048])
            # scalar queue (ACT idle early): fused KQ weights first
            # (k-th tile lands just ahead of the k-th accumulation
            # step; the tiny bias tiles would delay wkq[0] ~0.6us per
            # trigger), then q-bias, then V weights
            for k in range(KC):
                nc.scalar.dma_start(wkq[k][:], wkq_d[128 * k:128 * (k + 1), :])
            for h in range(HPC):
                nc.scalar.dma_start(bqt[h][:], bq_d[64 * h:64 * (h + 1), :])
            for k in range(KC):
                nc.scalar.dma_start(wv[k][:], wv_d[128 * k:128 * (k + 1), :])

            # wp is only needed by the projection tail; queue it after
            # the x^T burst
            for t in range(2):
                nc.sync.dma_start(wp[t][:], wp_d[128 * t:128 * (t + 1), :])

            with tc.tile_pool(name="ps2", bufs=1, space="PSUM") as ps2:
                # all front work (warmup, QKV projections, V chunks)
                # borrows "s" PSUM slots, so half of it can interleave
                # into the first attention pair while the second x^T
                # halves are still streaming in
                for i in range(NWARM):
                    ps = ps2.tile([128, NBLK], f32, tag="s", bufs=3,
                                  name=f"warm{i}")
                    nc.tensor.matmul(ps[:, 0:512], warm_t[:, 0:128],
                                     warm_t[:])

                def g_group(b, g):
                    # g0: K_h0,K_h1 / g1: K_h2,Q_h0 / g2: Q_h1,Q_h2
                    nsl = slice(512 * b, 512 * (b + 1))
                    xh = xtf if b < 2 else xts
                    hsl = slice(512 * (b % 2), 512 * (b % 2 + 1))
                    pf = ps2.tile([128, NBLK], f32, tag="s", bufs=3,
                                  name="ps_qk")
                    ps = pf[:, 0:512]
                    for k in range(KC):
                        nc.tensor.matmul(
                            ps, wkq[k][:, 128 * g:128 * (g + 1)],
                            xh[k][:, hsl],
                            start=(k == 0), stop=(k == KC - 1))
                    if g == 0:
                        nc.vector.tensor_copy(kh[0][0:64, nsl], ps[0:64, :])
                        nc.vector.tensor_copy(kh[1][64:128, nsl],
                                              ps[64:128, :])
                    elif g == 1:
                        nc.vector.tensor_copy(kh[2][0:64, nsl], ps[0:64, :])
                        nc.vector.tensor_scalar(
                            qA[0:64, nsl], ps[64:128, :], 0.125,
                            bqt[0][:], mult, add)
                    else:
                        nc.vector.tensor_scalar(
                            qA[64:128, nsl], ps[0:64, :], 0.125,
                            bqt[1][:], mult, add)
                        nc.vector.tensor_scalar(
                            qB[0:64, nsl], ps[64:128, :], 0.125,
                            bqt[2][:], mult, add)

                def v_chunk(m):
                    xh = xtf if m < 8 else xts
                    msl = slice(128 * (m % 8), 128 * (m % 8 + 1))
                    pf = ps2.tile([128, NBLK], f32, tag="s", bufs=3,
                                  name="ps_v")
                    ps = pf[:, 0:512]
                    for k in range(KC):
                        nc.tensor.matmul(ps[:, 0:J], xh[k][:, msl], wv[k][:],
                                         start=(k == 0), stop=(k == KC - 1))
                    vdst = vx[m].rearrange("p (h e) -> p h e",
                                           e=65)[:, :, 0:64]
                    nc.vector.tensor_copy(
                        vdst, ps[:, 0:J].rearrange("p (h e) -> p h e", e=64))

                pend = []

                def av_flush():
                    avh, h, nb, m, e = pend.pop(0)
                    vsl = slice(65 * h, 65 * (h + 1))
                    for i in range(2):
                        nc.tensor.matmul(
                            avh[i][:], vx[m][:, vsl],
                            e[:, 512 * i:512 * (i + 1)],
                            start=(m == 0), stop=(m == MC - 1))
                    if m != MC - 1:
                        return
                    adst, r0 = ((ah2[0], 0) if h == 0 else
                                (ah2[0], 64) if h == 1 else
                                (ah2[1], 0))
                    # raw copies first — they alone gate the next pair's
                    # AV PSUM writes; on ACT so the DVE exp stream is
                    # not disturbed. Sums rows and the broadcast/
                    # reciprocal/multiply chains follow off the critical
                    # path
                    for i in range(2):
                        hf = slice(NBLK * nb + 512 * i,
                                   NBLK * nb + 512 * (i + 1))
                        nc.scalar.activation(raw[h][:, hf],
                                             avh[i][0:64, :], Copy)
                    for i in range(2):
                        hf = slice(NBLK * nb + 512 * i,
                                   NBLK * nb + 512 * (i + 1))
                        nc.vector.tensor_copy(sums[h][:, hf],
                                              avh[i][64:65, :])
                    for i in range(2):
                        hf = slice(NBLK * nb + 512 * i,
                                   NBLK * nb + 512 * (i + 1))
                        bcs = pbc.tile([64, 512], f32, tag="bcs",
                                       name="bcs")
                        nc.gpsimd.partition_broadcast(bcs[:],
                                                      sums[h][:, hf])
                        rec = pbc.tile([64, 512], f32, tag="rec",
                                       name="rec")
                        nc.vector.reciprocal_approx_fast(rec[:], bcs[:])
                        nc.vector.tensor_mul(adst[r0:r0 + 64, hf],
                                             raw[h][:, hf], rec[:])

                def attn_pair(h, nb, extra=None):
                    # extra: dict chunk -> [thunks] fired after that
                    # chunk (interleaved projection / late front work)
                    qt = qA if h < 2 else qB
                    avh = [ps2.tile([65, 512], f32, tag=f"av{i}", bufs=1,
                                    name=f"ps_av{i}") for i in range(2)]
                    for m in range(MC):
                        msl = slice(128 * m, 128 * (m + 1))
                        s = ps2.tile([128, NBLK], f32, tag="s", bufs=3,
                                     name="ps_s")
                        for i in range(2):
                            q0 = NBLK * nb + 512 * i
                            nc.tensor.matmul(
                                s[:, 512 * i:512 * (i + 1)],
                                kh[h][:, msl], qt[:, q0:q0 + 512])
                        e = pe.tile([128, NBLK], bf16, tag="e", name="e")
                        if m % 2 == 1:
                            # odd chunks: Schraudolph exp on DVE
                            # (int16 bit trick, ~3% elementwise error
                            # that mostly cancels in softmax) to keep
                            # the ACT engine off the critical path
                            nc.vector.tensor_scalar(
                                e.bitcast(i16)[:], s[:],
                                SCH_A, SCH_B, mult, add)
                        else:
                            nc.scalar.activation(e[:], s[:], Exp)
                        pend.append((avh, h, nb, m, e))
                        if len(pend) > 4:
                            av_flush()
                        if extra and m in extra:
                            for th in extra[m]:
                                th()

                def proj(m):
                    # output projection chunk, borrowing an "s" PSUM
                    # slot so it can interleave into attention pairs;
                    # PSUM evacuation split across ACT and DVE
                    msl = slice(128 * m, 128 * (m + 1))
                    proj1(m)
                    proj2(m)

                pjt = {}

                def proj1(m):
                    # ah2[1] (head 2) part first: it is flushed two
                    # pairs before ah2[0], so these matmuls can run
                    # while the last pair's normalize chain completes
                    msl = slice(128 * m, 128 * (m + 1))
                    t = ps2.tile([128, NBLK], f32, tag="s", bufs=3,
                                 name="ps_pj")
                    pjt[m] = t
                    nc.tensor.matmul(t[:, 0:512], ah2[1][:, msl],
                                     wp[1][:, 0:512],
                                     start=True, stop=False)
                    nc.tensor.matmul(t[:, 512:768], ah2[1][:, msl],
                                     wp[1][:, 512:768],
                                     start=True, stop=False)

                def proj2(m):
                    msl = slice(128 * m, 128 * (m + 1))
                    t = pjt.pop(m)
                    nc.tensor.matmul(t[:, 0:512], ah2[0][:, msl],
                                     wp[0][:, 0:512],
                                     start=False, stop=True)
                    nc.tensor.matmul(t[:, 512:768], ah2[0][:, msl],
                                     wp[0][:, 512:768],
                                     start=False, stop=True)
                    o3 = posb.tile([128, C], bf16, tag="o3", name="o3")
                    nc.vector.tensor_copy(o3[:, 0:512], t[:, 0:512])
                    nc.scalar.activation(o3[:, 512:768], t[:, 512:768],
                                         Copy)
                    # rotate output queues so the final DMA drain
                    # is not serialized on one ring
                    oq = [nc.sync, nc.scalar, nc.gpsimd][m % 3]
                    oq.dma_start(out_d[msl, :], o3[:])

                def gg(b, g):
                    return lambda: g_group(b, g)

                def vv(m):
                    return lambda: v_chunk(m)

                def pj(m):
                    return lambda: proj(m)

                # minimal pre-pair front: only what pair (2,0) chunks
                # 0..7 need (kh2 + qB over the first x^T halves).
                # Everything else - kh0/kh1, b2/b3 projections, all V
                # chunks - rides inside pair (2,0), overlapping the
                # second x^T halves still streaming in. Block-0
                # output projection rides inside the block-1 pairs,
                # so only 8 proj chunks and the last flush remain in
                # the serial tail.
                for b in (0, 1):
                    g_group(b, 1)
                for b in (0, 1):
                    g_group(b, 2)
                ext0 = {0: [vv(0), gg(0, 0)], 1: [vv(1), gg(1, 0)],
                        2: [vv(2)], 3: [vv(3)],
                        4: [vv(4), gg(2, 1)], 5: [vv(5), gg(3, 1)],
                        6: [vv(6), gg(2, 0)], 7: [vv(7), gg(3, 0)],
                        8: [vv(8), gg(2, 2)], 9: [vv(9), gg(3, 2)]}
                for m in range(10, MC):
                    ext0[m] = [vv(m)]
                attn_pair(2, 0, ext0)
                attn_pair(0, 0)
                attn_pair(1, 0)
                # block-0 proj spread thinly over all block-1 pairs
                # ((2,1) only from chunk 8, after (1,0)'s normalize)
                attn_pair(2, 1, {8: [pj(0)], 11: [pj(1)]})
                attn_pair(0, 1, {4: [pj(2)], 8: [pj(3)], 12: [pj(4)]})
                attn_pair(1, 1, {4: [pj(5)], 8: [pj(6)], 12: [pj(7)]})
                while pend:
                    av_flush()
                # tail: run the ah2[1] halves of the first three
                # proj chunks under the last flush's normalize chain,
                # then pipeline phase1/phase2
                for m in range(8, 11):
                    proj1(m)
                for m in range(8, MC):
                    proj2(m)
                    if m + 3 < MC:
                        proj1(m + 3)

    nc.compile()
    return nc


def kernel(x, w_qkv, b_qkv, w_proj, b_proj):
    import ml_dtypes

    from concourse.bass_utils import run_bass_kernel_spmd

    global LAST_RESULTS
    if "nc" not in _cache:
        _cache["nc"] = _build()
    nc = _cache["nc"]

    bf16 = ml_dtypes.bfloat16
    x = np.asarray(x, dtype=np.float32)
    w_qkv = np.asarray(w_qkv, dtype=np.float32)
    b_qkv = np.asarray(b_qkv, dtype=np.float32)
    w_proj = np.asarray(w_proj, dtype=np.float32)
    b_proj = np.asarray(b_proj, dtype=np.float32)

    in_maps = []
    for c in range(NCORES):
        b = c // 4
        h0 = HPC * (c % 4)
        qs = slice(64 * h0, 64 * (h0 + HPC))
        ks = slice(C + 64 * h0, C + 64 * (h0 + HPC))
        vs = slice(2 * C + 64 * h0, 2 * C + 64 * (h0 + HPC))
        wkq = np.concatenate([w_qkv[:, ks], w_qkv[:, qs]], axis=1)
        wp_pad = np.zeros((2 * 128, C), dtype=np.float32)
        wp_pad[0:128] = w_proj[64 * h0:64 * (h0 + 2), :]
        wp_pad[128:192] = w_proj[64 * (h0 + 2):64 * (h0 + 3), :]
        in_maps.append({
            "xt": np.ascontiguousarray(x[b].T).astype(bf16),
            "wkq": np.ascontiguousarray(wkq).astype(bf16),
            "wv": np.ascontiguousarray(w_qkv[:, vs]).astype(bf16),
            "bq": np.ascontiguousarray(
                (b_qkv[qs] * 0.125).reshape(J, 1)).astype(np.float32),
            "wp": wp_pad.astype(bf16),
        })

    res = run_bass_kernel_spmd(nc, in_maps, core_ids=list(range(NCORES)))
    LAST_RESULTS = res

    out = np.zeros((B, N, C), dtype=np.float32)
    for c in range(NCORES):
        out[c // 4] += np.asarray(res.results[c]["out"], dtype=np.float32)
    out += b_proj + b_qkv[2 * C:] @ w_proj
    return out



# revision 64
# speedup vs baseline: 1.0389x; 1.0068x over previous
"""Multi-head attention block on 8 TRN2 NeuronCores.

Problem: x[2,2048,768] -> qkv proj -> 12-head attention -> out proj.
Sharding: 24 (batch, head) pairs across 8 cores; core c handles batch
c//4 and heads 3*(c%4)..3*(c%4)+2. Each core computes its heads'
Q,K,V, attention, and a partial output projection; the host sums the
four per-batch partials and adds the bias terms.

Design notes (v8, ~169us vs v6's ~176us):
  - Exp split across engines: odd key chunks compute exp on the DVE
    via a Schraudolph bit trick (one tensor_scalar:
    int16(s*184.665 + 16249) bitcast to bf16; the ~3% elementwise
    error mostly cancels in softmax, rel err 1.07e-2 vs the 2e-2
    gate). This halves the ACT exp stream, making the attention
    inner loop PE-bound (~98% TensorE occupancy) instead of
    ACT-bound. Pair-end raw copies move to ACT for the same reason.
  - One PSUM pool for the whole kernel: s tiles (3 bufs, 6 banks) +
    two AV accumulators (2 banks). Front work (QKV projections, V
    chunks) and the output projection borrow s slots, so they can
    interleave anywhere in the attention stream.
  - Minimal pre-pair front: only kh2/qB over the first x^T halves
    runs before attention; kh0/kh1, the b2/b3 projections and all V
    chunks ride inside pair (2,0) while the second x^T halves
    stream in (separate first/second-half x^T tiles keep the
    dependencies decoupled). Attention starts ~33us in (was ~47us).
  - Pair order (2,0),(0,0),(1,0),(2,1),(0,1),(1,1); the block-0
    output projection rides inside the block-1 pairs, and the tail
    runs the ah2[1] halves of the first three remaining proj chunks
    under the last flush's normalize chain (proj accumulates head 2
    first for this reason). Output DMA alternates sync/scalar
    queues so the final drain is not serialized.
  - All matmul operands bf16; output bf16; host sums partials in
    f32. K/Q fused weight stream, packed Q tiles, zero-carrying K
    tiles, ones-column V tiles for the softmax sums, DVE-memset
    constants and PE warmup matmuls carried over from v6.
  - Things measured NOT to help on this hardware: tile_position row
    tiling for the 64-deep QK contraction (no MM concurrency, and
    half-array matmuls re-throttle the HAM clock gate to 1.2GHz),
    1024-column moving matmuls (ISA caps at 512), xt DMA on the
    gpsimd queue, gpsimd tensor ops beyond partition_broadcast
    (multi-us library reloads), fp8 DoubleRow AV (needs fp8 exp
    output, which blows the error budget).
"""

import os
import sys

for _p in ("/opt/trn_rl_repo", "/opt/pypackages"):
    if _p not in sys.path:
        sys.path.append(_p)

import numpy as np

B, N, C = 2, 2048, 768
H, D = 12, 64
HPC = 3                    # heads per core
J = HPC * D                # 192 per-core head-dim rows
NCORES = 8
NBLK = 1024                # query-block width (one exp per [128, NBLK])
NB = N // NBLK             # 2
MC = N // 128              # 16 key chunks
KC = C // 128              # 6 contraction chunks for projections
NWARM = 12

SCH_A = 184.6649652337873   # 2^7 / ln 2
SCH_B = 16249.0             # exponent-bias offset, tuned for softmax

_cache = {}
LAST_RESULTS = None


def _build():
    import concourse.mybir as mybir
    import concourse.tile as tile
    from concourse import bacc

    f32 = mybir.dt.float32
    bf16 = mybir.dt.bfloat16
    i16 = mybir.dt.int16
    Exp = mybir.ActivationFunctionType.Exp
    Copy = mybir.ActivationFunctionType.Copy
    mult = mybir.AluOpType.mult
    add = mybir.AluOpType.add

    nc = bacc.Bacc("TRN2", target_bir_lowering=False, debug=False,
                   num_devices=NCORES)

    xt_d = nc.declare_dram_parameter("xt", [C, N], bf16, isOutput=False)
    # fused [K_h0|K_h1|K_h2|Q_h0|Q_h1|Q_h2] weight columns
    wkq_d = nc.declare_dram_parameter("wkq", [C, 2 * J], bf16,
                                      isOutput=False)
    wv_d = nc.declare_dram_parameter("wv", [C, J], bf16, isOutput=False)
    bq_d = nc.declare_dram_parameter("bq", [J, 1], f32, isOutput=False)
    # padded proj weights: rows 0:128 = heads 0,1; 128:192 = head 2;
    # 192:256 = zero (annihilates ah2[1]'s junk bottom half)
    wp_d = nc.declare_dram_parameter("wp", [2 * 128, C], bf16,
                                     isOutput=False)
    out_d = nc.declare_dram_parameter("out", [N, C], bf16, isOutput=True)

    with tile.TileContext(nc) as tc:
        with (
            tc.tile_pool(name="persist", bufs=1) as pp,
            tc.tile_pool(name="osb", bufs=4) as posb,
            tc.tile_pool(name="etile", bufs=10) as pe,
            tc.tile_pool(name="bcsb", bufs=2) as pbc,
        ):
            warm_t = pp.tile([128, 512], bf16, tag="warm_t", name="warm_t")
            wkq = [pp.tile([128, 2 * J], bf16, tag=f"wkq{k}",
                           name=f"wkq{k}") for k in range(KC)]
            # x^T halves as separate tiles so first-half
            # consumers never wait on the second-half DMA
            xtf = [pp.tile([128, 1024], bf16, tag=f"xtf{k}",
                           name=f"xtf{k}") for k in range(KC)]
            xts = [pp.tile([128, 1024], bf16, tag=f"xts{k}",
                           name=f"xts{k}") for k in range(KC)]
            wv = [pp.tile([128, J], bf16, tag=f"wv{k}", name=f"wv{k}")
                  for k in range(KC)]
            bqt = [pp.tile([64, 1], f32, tag=f"bq{h}", name=f"bq{h}")
                   for h in range(HPC)]
            wp = [pp.tile([128, C], bf16, tag=f"wp{t}", name=f"wp{t}")
                  for t in range(2)]
            # K^T per head, zero rows alternating so the packed Q tiles
            # need none: kh0 data 0:64, kh1 data 64:128, kh2 data 0:64
            kh = [pp.tile([128, N], bf16, tag=f"kh{h}", name=f"kh{h}")
                  for h in range(HPC)]
            # Q^T packed: qA = (q0 top, q1 bottom); qB = (q2 top, junk)
            qA = pp.tile([128, N], bf16, tag="qA", name="qA")
            qB = pp.tile([128, N], bf16, tag="qB", name="qB")
            # V with a ones column per head: [128, 3*65]
            vx = [pp.tile([128, HPC * 65], bf16, tag=f"vx{m}",
                          name=f"vx{m}") for m in range(MC)]
            sums = [pp.tile([1, N], f32, tag=f"sums{h}", name=f"sums{h}")
                    for h in range(HPC)]
            # unnormalized attention outputs (release AV PSUM fast),
            # base partition 0 so tensor_mul's SBUF inputs stay aligned
            raw = [pp.tile([64, N], bf16, tag=f"raw{h}", name=f"raw{h}")
                   for h in range(HPC)]
            ah2 = [pp.tile([128, N], bf16, tag=f"ah2{t}", name=f"ah2{t}")
                   for t in range(2)]

            # ---- constants via DVE memset (no DMA traffic; gpsimd
            # memset on partition-offset APs crashed the device) ----
            nc.vector.memset(warm_t[:], 1.0)
            nc.vector.memset(kh[0][64:128, :], 0.0)
            nc.vector.memset(kh[1][0:64, :], 0.0)
            nc.vector.memset(kh[2][64:128, :], 0.0)
            nc.vector.memset(qB[64:128, :], 0.0)
            nc.vector.memset(ah2[1][64:128, :], 0.0)
            for m in range(MC):
                on = vx[m].rearrange("p (h e) -> p h e", e=65)[:, :, 64:65]
                nc.vector.memset(on, 1.0)

            # ---- input DMA, ordered for earliest compute start ----
            # sync queue: x^T half-row tiles only (the long pole; each
            # extra trigger ahead of an xt tile delays it ~0.6us), first
            # halves feed blocks 0/1, then second halves
            for k in range(KC):
                nc.sync.dma_start(xtf[k][:],
                                  xt_d[128 * k:128 * (k + 1), 0:1024])
            for k in range(KC):
                nc.sync.dma_start(xts[k][:],
                                  xt_d[128 * k:128 * (k + 1), 1024:2048])
            # scalar queue (ACT idle early): fused KQ weights first
            # (k-th tile lands just ahead of the k-th accumulation
            # step; the tiny bias tiles would delay wkq[0] ~0.6us per
            # trigger), then q-bias, then V weights
            for k in range(KC):
                nc.scalar.dma_start(wkq[k][:], wkq_d[128 * k:128 * (k + 1), :])
            for h in range(HPC):
                nc.scalar.dma_start(bqt[h][:], bq_d[64 * h:64 * (h + 1), :])
            for k in range(KC):
                nc.scalar.dma_start(wv[k][:], wv_d[128 * k:128 * (k + 1), :])

            # wp is only needed by the projection tail; queue it after
            # the x^T burst
            for t in range(2):
                nc.sync.dma_start(wp[t][:], wp_d[128 * t:128 * (t + 1), :])

            with tc.tile_pool(name="ps2", bufs=1, space="PSUM") as ps2:
                # all front work (warmup, QKV projections, V chunks)
                # borrows "s" PSUM slots, so half of it can interleave
                # into the first attention pair while the second x^T
                # halves are still streaming in
                for i in range(NWARM):
                    ps = ps2.tile([128, NBLK], f32, tag="s", bufs=3,
                                  name=f"warm{i}")
                    nc.tensor.matmul(ps[:, 0:512], warm_t[:, 0:128],
                                     warm_t[:])

                def g_group(b, g):
                    # g0: K_h0,K_h1 / g1: K_h2,Q_h0 / g2: Q_h1,Q_h2
                    nsl = slice(512 * b, 512 * (b + 1))
                    xh = xtf if b < 2 else xts
                    hsl = slice(512 * (b % 2), 512 * (b % 2 + 1))
                    pf = ps2.tile([128, NBLK], f32, tag="s", bufs=3,
                                  name="ps_qk")
                    ps = pf[:, 0:512]
                    for k in range(KC):
                        nc.tensor.matmul(
                            ps, wkq[k][:, 128 * g:128 * (g + 1)],
                            xh[k][:, hsl],
                            start=(k == 0), stop=(k == KC - 1))
                    if g == 0:
                        nc.vector.tensor_copy(kh[0][0:64, nsl], ps[0:64, :])
                        nc.vector.tensor_copy(kh[1][64:128, nsl],
                                              ps[64:128, :])
                    elif g == 1:
                        nc.vector.tensor_copy(kh[2][0:64, nsl], ps[0:64, :])
                        nc.vector.tensor_scalar(
                            qA[0:64, nsl], ps[64:128, :], 0.125,
                            bqt[0][:], mult, add)
                    else:
                        nc.vector.tensor_scalar(
                            qA[64:128, nsl], ps[0:64, :], 0.125,
                            bqt[1][:], mult, add)
                        nc.vector.tensor_scalar(
                            qB[0:64, nsl], ps[64:128, :], 0.125,
                            bqt[2][:], mult, add)

                def v_chunk(m):
                    xh = xtf if m < 8 else xts
                    msl = slice(128 * (m % 8), 128 * (m % 8 + 1))
                    pf = ps2.tile([128, NBLK], f32, tag="s", bufs=3,
                                  name="ps_v")
                    ps = pf[:, 0:512]
                    for k in range(KC):
                        nc.tensor.matmul(ps[:, 0:J], xh[k][:, msl], wv[k][:],
                                         start=(k == 0), stop=(k == KC - 1))
                    vdst = vx[m].rearrange("p (h e) -> p h e",
                                           e=65)[:, :, 0:64]
                    nc.vector.tensor_copy(
                        vdst, ps[:, 0:J].rearrange("p (h e) -> p h e", e=64))

                pend = []

                def av_flush():
                    avh, h, nb, m, e = pend.pop(0)
                    vsl = slice(65 * h, 65 * (h + 1))
                    for i in range(2):
                        nc.tensor.matmul(
                            avh[i][:], vx[m][:, vsl],
                            e[:, 512 * i:512 * (i + 1)],
                            start=(m == 0), stop=(m == MC - 1))
                    if m != MC - 1:
                        return
                    adst, r0 = ((ah2[0], 0) if h == 0 else
                                (ah2[0], 64) if h == 1 else
                                (ah2[1], 0))
                    # raw copies first — they alone gate the next pair's
                    # AV PSUM writes; on ACT so the DVE exp stream is
                    # not disturbed. Sums rows and the broadcast/
                    # reciprocal/multiply chains follow off the critical
                    # path
                    for i in range(2):
                        hf = slice(NBLK * nb + 512 * i,
                                   NBLK * nb + 512 * (i + 1))
                        nc.scalar.activation(raw[h][:, hf],
                                             avh[i][0:64, :], Copy)
                    for i in range(2):
                        hf = slice(NBLK * nb + 512 * i,
                                   NBLK * nb + 512 * (i + 1))
                        nc.vector.tensor_copy(sums[h][:, hf],
                                              avh[i][64:65, :])
                    for i in range(2):
                        hf = slice(NBLK * nb + 512 * i,
                                   NBLK * nb + 512 * (i + 1))
                        bcs = pbc.tile([64, 512], f32, tag="bcs",
                                       name="bcs")
                        nc.gpsimd.partition_broadcast(bcs[:],
                                                      sums[h][:, hf])
                        rec = pbc.tile([64, 512], f32, tag="rec",
                                       name="rec")
                        nc.vector.reciprocal_approx_fast(rec[:], bcs[:])
                        nc.vector.tensor_mul(adst[r0:r0 + 64, hf],
                                             raw[h][:, hf], rec[:])

                def attn_pair(h, nb, extra=None):
                    # extra: dict chunk -> [thunks] fired after that
                    # chunk (interleaved projection / late front work)
                    qt = qA if h < 2 else qB
                    avh = [ps2.tile([65, 512], f32, tag=f"av{i}", bufs=1,
                                    name=f"ps_av{i}") for i in range(2)]
                    for m in range(MC):
                        msl = slice(128 * m, 128 * (m + 1))
                        s = ps2.tile([128, NBLK], f32, tag="s", bufs=3,
                                     name="ps_s")
                        for i in range(2):
                            q0 = NBLK * nb + 512 * i
                            nc.tensor.matmul(
                                s[:, 512 * i:512 * (i + 1)],
                                kh[h][:, msl], qt[:, q0:q0 + 512])
                        e = pe.tile([128, NBLK], bf16, tag="e", name="e")
                        if m % 2 == 1:
                            # odd chunks: Schraudolph exp on DVE
                            # (int16 bit trick, ~3% elementwise error
                            # that mostly cancels in softmax) to keep
                            # the ACT engine off the critical path
                            nc.vector.tensor_scalar(
                                e.bitcast(i16)[:], s[:],
                                SCH_A, SCH_B, mult, add)
                        else:
                            nc.scalar.activation(e[:], s[:], Exp)
                        pend.append((avh, h, nb, m, e))
                        if len(pend) > 5:
                            av_flush()
                        if extra and m in extra:
                            for th in extra[m]:
                                th()

                def proj(m):
                    # output projection chunk, borrowing an "s" PSUM
                    # slot so it can interleave into attention pairs;
                    # PSUM evacuation split across ACT and DVE
                    msl = slice(128 * m, 128 * (m + 1))
                    proj1(m)
                    proj2(m)

                pjt = {}

                def proj1(m):
                    # ah2[1] (head 2) part first: it is flushed two
                    # pairs before ah2[0], so these matmuls can run
                    # while the last pair's normalize chain completes
                    msl = slice(128 * m, 128 * (m + 1))
                    t = ps2.tile([128, NBLK], f32, tag="s", bufs=3,
                                 name="ps_pj")
                    pjt[m] = t
                    nc.tensor.matmul(t[:, 0:512], ah2[1][:, msl],
                                     wp[1][:, 0:512],
                                     start=True, stop=False)
                    nc.tensor.matmul(t[:, 512:768], ah2[1][:, msl],
                                     wp[1][:, 512:768],
                                     start=True, stop=False)

                def proj2(m):
                    msl = slice(128 * m, 128 * (m + 1))
                    t = pjt.pop(m)
                    nc.tensor.matmul(t[:, 0:512], ah2[0][:, msl],
                                     wp[0][:, 0:512],
                                     start=False, stop=True)
                    nc.tensor.matmul(t[:, 512:768], ah2[0][:, msl],
                                     wp[0][:, 512:768],
                                     start=False, stop=True)
                    o3 = posb.tile([128, C], bf16, tag="o3", name="o3")
                    nc.vector.tensor_copy(o3[:, 0:512], t[:, 0:512])
                    nc.scalar.activation(o3[:, 512:768], t[:, 512:768],
                                         Copy)
                    # rotate output queues so the final DMA drain
                    # is not serialized on one ring
                    oq = [nc.sync, nc.scalar, nc.gpsimd][m % 3]
                    oq.dma_start(out_d[msl, :], o3[:])

                def gg(b, g):
                    return lambda: g_group(b, g)

                def vv(m):
                    return lambda: v_chunk(m)

                def pj(m):
                    return lambda: proj(m)

                # minimal pre-pair front: only what pair (2,0) chunks
                # 0..7 need (kh2 + qB over the first x^T halves).
                # Everything else - kh0/kh1, b2/b3 projections, all V
                # chunks - rides inside pair (2,0), overlapping the
                # second x^T halves still streaming in. Block-0
                # output projection rides inside the block-1 pairs,
                # so only 8 proj chunks and the last flush remain in
                # the serial tail.
                for b in (0, 1):
                    g_group(b, 1)
                for b in (0, 1):
                    g_group(b, 2)
                ext0 = {0: [vv(0), gg(0, 0)], 1: [vv(1), gg(1, 0)],
                        2: [vv(2)], 3: [vv(3)],
                        4: [vv(4), gg(2, 1)], 5: [vv(5), gg(3, 1)],
                        6: [vv(6), gg(2, 0)], 7: [vv(7), gg(3, 0)],
                        8: [vv(8), gg(2, 2)], 9: [vv(9), gg(3, 2)]}
                for m in range(10, MC):
                    ext0[m] = [vv(m)]
                attn_pair(2, 0, ext0)
                attn_pair(0, 0)
                attn_pair(1, 0)
                # block-0 proj spread thinly over all block-1 pairs
                # ((2,1) only from chunk 8, after (1,0)'s normalize)
                attn_pair(2, 1, {8: [pj(0)], 11: [pj(1)]})
                attn_pair(0, 1, {4: [pj(2)], 8: [pj(3)], 12: [pj(4)]})
                attn_pair(1, 1, {4: [pj(5)], 8: [pj(6)], 12: [pj(7)]})
                while pend:
                    av_flush()
                # tail: run the ah2[1] halves of the first three
                # proj chunks under the last flush's normalize chain,
                # then pipeline phase1/phase2
                for m in range(8, 11):
                    proj1(m)
                for m in range(8, MC):
                    proj2(m)
                    if m + 3 < MC:
                        proj1(m + 3)

    nc.compile()
    return nc


def kernel(x, w_qkv, b_qkv, w_proj, b_proj):
    import ml_dtypes

    from concourse.bass_utils import run_bass_kernel_spmd

    global LAST_RESULTS
    if "nc" not in _cache:
        _cache["nc"] = _build()
    nc = _cache["nc"]

    bf16 = ml_dtypes.bfloat16
    x = np.asarray(x, dtype=np.float32)
    w_qkv = np.asarray(w_qkv, dtype=np.float32)
    b_qkv = np.asarray(b_qkv, dtype=np.float32)
    w_proj = np.asarray(w_proj, dtype=np.float32)
    b_proj = np.asarray(b_proj, dtype=np.float32)

    in_maps = []
    for c in range(NCORES):
        b = c // 4
        h0 = HPC * (c % 4)
        qs = slice(64 * h0, 64 * (h0 + HPC))
        ks = slice(C + 64 * h0, C + 64 * (h0 + HPC))
        vs = slice(2 * C + 64 * h0, 2 * C + 64 * (h0 + HPC))
        wkq = np.concatenate([w_qkv[:, ks], w_qkv[:, qs]], axis=1)
        wp_pad = np.zeros((2 * 128, C), dtype=np.float32)
        wp_pad[0:128] = w_proj[64 * h0:64 * (h0 + 2), :]
        wp_pad[128:192] = w_proj[64 * (h0 + 2):64 * (h0 + 3), :]
        in_maps.append({
            "xt": np.ascontiguousarray(x[b].T).astype(bf16),
            "wkq": np.ascontiguousarray(wkq).astype(bf16),
            "wv": np.ascontiguousarray(w_qkv[:, vs]).astype(bf16),
            "bq": np.ascontiguousarray(
                (b_qkv[qs] * 0.125).reshape(J, 1)).astype(np.float32),
            "wp": wp_pad.astype(bf16),
        })

    res = run_bass_kernel_spmd(nc, in_maps, core_ids=list(range(NCORES)))
    LAST_RESULTS = res

    out = np.zeros((B, N, C), dtype=np.float32)
    for c in range(NCORES):
        out[c // 4] += np.asarray(res.results[c]["out"], dtype=np.float32)
    out += b_proj + b_qkv[2 * C:] @ w_proj
    return out



# revision 65
# speedup vs baseline: 1.0414x; 1.0024x over previous
"""Multi-head attention block on 8 TRN2 NeuronCores.

Problem: x[2,2048,768] -> qkv proj -> 12-head attention -> out proj.
Sharding: 24 (batch, head) pairs across 8 cores; core c handles batch
c//4 and heads 3*(c%4)..3*(c%4)+2. Each core computes its heads'
Q,K,V, attention, and a partial output projection; the host sums the
four per-batch partials and adds the bias terms.

Design notes (v8, ~169us vs v6's ~176us):
  - Exp split across engines: odd key chunks compute exp on the DVE
    via a Schraudolph bit trick (one tensor_scalar:
    int16(s*184.665 + 16249) bitcast to bf16; the ~3% elementwise
    error mostly cancels in softmax, rel err 1.07e-2 vs the 2e-2
    gate). This halves the ACT exp stream, making the attention
    inner loop PE-bound (~98% TensorE occupancy) instead of
    ACT-bound. Pair-end raw copies move to ACT for the same reason.
  - One PSUM pool for the whole kernel: s tiles (3 bufs, 6 banks) +
    two AV accumulators (2 banks). Front work (QKV projections, V
    chunks) and the output projection borrow s slots, so they can
    interleave anywhere in the attention stream.
  - Minimal pre-pair front: only kh2/qB over the first x^T halves
    runs before attention; kh0/kh1, the b2/b3 projections and all V
    chunks ride inside pair (2,0) while the second x^T halves
    stream in (separate first/second-half x^T tiles keep the
    dependencies decoupled). Attention starts ~33us in (was ~47us).
  - Pair order (2,0),(0,0),(1,0),(2,1),(0,1),(1,1); the block-0
    output projection rides inside the block-1 pairs, and the tail
    runs the ah2[1] halves of the first three remaining proj chunks
    under the last flush's normalize chain (proj accumulates head 2
    first for this reason). Output DMA alternates sync/scalar
    queues so the final drain is not serialized.
  - All matmul operands bf16; output bf16; host sums partials in
    f32. K/Q fused weight stream, packed Q tiles, zero-carrying K
    tiles, ones-column V tiles for the softmax sums, DVE-memset
    constants and PE warmup matmuls carried over from v6.
  - Things measured NOT to help on this hardware: tile_position row
    tiling for the 64-deep QK contraction (no MM concurrency, and
    half-array matmuls re-throttle the HAM clock gate to 1.2GHz),
    1024-column moving matmuls (ISA caps at 512), xt DMA on the
    gpsimd queue, gpsimd tensor ops beyond partition_broadcast
    (multi-us library reloads), fp8 DoubleRow AV (needs fp8 exp
    output, which blows the error budget).
"""

import os
import sys

for _p in ("/opt/trn_rl_repo", "/opt/pypackages"):
    if _p not in sys.path:
        sys.path.append(_p)

import numpy as np

B, N, C = 2, 2048, 768
H, D = 12, 64
HPC = 3                    # heads per core
J = HPC * D                # 192 per-core head-dim rows
NCORES = 8
NBLK = 1024                # query-block width (one exp per [128, NBLK])
NB = N // NBLK             # 2
MC = N // 128              # 16 key chunks
KC = C // 128              # 6 contraction chunks for projections
NWARM = 12

SCH_A = 184.6649652337873   # 2^7 / ln 2
SCH_B = 16249.0             # exponent-bias offset, tuned for softmax

_cache = {}
LAST_RESULTS = None


def _build():
    import concourse.mybir as mybir
    import concourse.tile as tile
    from concourse import bacc

    f32 = mybir.dt.float32
    bf16 = mybir.dt.bfloat16
    i16 = mybir.dt.int16
    Exp = mybir.ActivationFunctionType.Exp
    Copy = mybir.ActivationFunctionType.Copy
    mult = mybir.AluOpType.mult
    add = mybir.AluOpType.add

    nc = bacc.Bacc("TRN2", target_bir_lowering=False, debug=False,
                   num_devices=NCORES)

    xt_d = nc.declare_dram_parameter("xt", [C, N], bf16, isOutput=False)
    # fused [K_h0|K_h1|K_h2|Q_h0|Q_h1|Q_h2] weight columns
    wkq_d = nc.declare_dram_parameter("wkq", [C, 2 * J], bf16,
                                      isOutput=False)
    wv_d = nc.declare_dram_parameter("wv", [C, J], bf16, isOutput=False)
    bq_d = nc.declare_dram_parameter("bq", [J, 1], f32, isOutput=False)
    # padded proj weights: rows 0:128 = heads 0,1; 128:192 = head 2;
    # 192:256 = zero (annihilates ah2[1]'s junk bottom half)
    wp_d = nc.declare_dram_parameter("wp", [2 * 128, C], bf16,
                                     isOutput=False)
    out_d = nc.declare_dram_parameter("out", [N, C], bf16, isOutput=True)

    with tile.TileContext(nc) as tc:
        with (
            tc.tile_pool(name="persist", bufs=1) as pp,
            tc.tile_pool(name="osb", bufs=4) as posb,
            tc.tile_pool(name="etile", bufs=12) as pe,
            tc.tile_pool(name="bcsb", bufs=2) as pbc,
        ):
            warm_t = pp.tile([128, 512], bf16, tag="warm_t", name="warm_t")
            wkq = [pp.tile([128, 2 * J], bf16, tag=f"wkq{k}",
                           name=f"wkq{k}") for k in range(KC)]
            # x^T halves as separate tiles so first-half
            # consumers never wait on the second-half DMA
            xtf = [pp.tile([128, 1024], bf16, tag=f"xtf{k}",
                           name=f"xtf{k}") for k in range(KC)]
            xts = [pp.tile([128, 1024], bf16, tag=f"xts{k}",
                           name=f"xts{k}") for k in range(KC)]
            wv = [pp.tile([128, J], bf16, tag=f"wv{k}", name=f"wv{k}")
                  for k in range(KC)]
            bqt = [pp.tile([64, 1], f32, tag=f"bq{h}", name=f"bq{h}")
                   for h in range(HPC)]
            wp = [pp.tile([128, C], bf16, tag=f"wp{t}", name=f"wp{t}")
                  for t in range(2)]
            # K^T per head, zero rows alternating so the packed Q tiles
            # need none: kh0 data 0:64, kh1 data 64:128, kh2 data 0:64
            kh = [pp.tile([128, N], bf16, tag=f"kh{h}", name=f"kh{h}")
                  for h in range(HPC)]
            # Q^T packed: qA = (q0 top, q1 bottom); qB = (q2 top, junk)
            qA = pp.tile([128, N], bf16, tag="qA", name="qA")
            qB = pp.tile([128, N], bf16, tag="qB", name="qB")
            # V with a ones column per head: [128, 3*65]
            vx = [pp.tile([128, HPC * 65], bf16, tag=f"vx{m}",
                          name=f"vx{m}") for m in range(MC)]
            sums = [pp.tile([1, N], f32, tag=f"sums{h}", name=f"sums{h}")
                    for h in range(HPC)]
            # unnormalized attention outputs (release AV PSUM fast),
            # base partition 0 so tensor_mul's SBUF inputs stay aligned
            raw = [pp.tile([64, N], bf16, tag=f"raw{h}", name=f"raw{h}")
                   for h in range(HPC)]
            ah2 = [pp.tile([128, N], bf16, tag=f"ah2{t}", name=f"ah2{t}")
                   for t in range(2)]

            # ---- constants via DVE memset (no DMA traffic; gpsimd
            # memset on partition-offset APs crashed the device) ----
            nc.vector.memset(warm_t[:], 1.0)
            nc.vector.memset(kh[0][64:128, :], 0.0)
            nc.vector.memset(kh[1][0:64, :], 0.0)
            nc.vector.memset(kh[2][64:128, :], 0.0)
            nc.vector.memset(qB[64:128, :], 0.0)
            nc.vector.memset(ah2[1][64:128, :], 0.0)
            for m in range(MC):
                on = vx[m].rearrange("p (h e) -> p h e", e=65)[:, :, 64:65]
                nc.vector.memset(on, 1.0)

            # ---- input DMA, ordered for earliest compute start ----
            # sync queue: x^T half-row tiles only (the long pole; each
            # extra trigger ahead of an xt tile delays it ~0.6us), first
            # halves feed blocks 0/1, then second halves
            for k in range(KC):
                nc.sync.dma_start(xtf[k][:],
                                  xt_d[128 * k:128 * (k + 1), 0:1024])
            for k in range(KC):
                nc.sync.dma_start(xts[k][:],
                                  xt_d[128 * k:128 * (k + 1), 1024:2048])
            # scalar queue (ACT idle early): fused KQ weights first
            # (k-th tile lands just ahead of the k-th accumulation
            # step; the tiny bias tiles would delay wkq[0] ~0.6us per
            # trigger), then q-bias, then V weights
            for k in range(KC):
                nc.scalar.dma_start(wkq[k][:], wkq_d[128 * k:128 * (k + 1), :])
            for h in range(HPC):
                nc.scalar.dma_start(bqt[h][:], bq_d[64 * h:64 * (h + 1), :])
            for k in range(KC):
                nc.scalar.dma_start(wv[k][:], wv_d[128 * k:128 * (k + 1), :])

            # wp is only needed by the projection tail; queue it after
            # the x^T burst
            for t in range(2):
                nc.sync.dma_start(wp[t][:], wp_d[128 * t:128 * (t + 1), :])

            with tc.tile_pool(name="ps2", bufs=1, space="PSUM") as ps2:
                # all front work (warmup, QKV projections, V chunks)
                # borrows "s" PSUM slots, so half of it can interleave
                # into the first attention pair while the second x^T
                # halves are still streaming in
                for i in range(NWARM):
                    ps = ps2.tile([128, NBLK], f32, tag="s", bufs=3,
                                  name=f"warm{i}")
                    nc.tensor.matmul(ps[:, 0:512], warm_t[:, 0:128],
                                     warm_t[:])

                def g_group(b, g):
                    # g0: K_h0,K_h1 / g1: K_h2,Q_h0 / g2: Q_h1,Q_h2
                    nsl = slice(512 * b, 512 * (b + 1))
                    xh = xtf if b < 2 else xts
                    hsl = slice(512 * (b % 2), 512 * (b % 2 + 1))
                    pf = ps2.tile([128, NBLK], f32, tag="s", bufs=3,
                                  name="ps_qk")
                    ps = pf[:, 0:512]
                    for k in range(KC):
                        nc.tensor.matmul(
                            ps, wkq[k][:, 128 * g:128 * (g + 1)],
                            xh[k][:, hsl],
                            start=(k == 0), stop=(k == KC - 1))
                    if g == 0:
                        nc.vector.tensor_copy(kh[0][0:64, nsl], ps[0:64, :])
                        nc.vector.tensor_copy(kh[1][64:128, nsl],
                                              ps[64:128, :])
                    elif g == 1:
                        nc.vector.tensor_copy(kh[2][0:64, nsl], ps[0:64, :])
                        nc.vector.tensor_scalar(
                            qA[0:64, nsl], ps[64:128, :], 0.125,
                            bqt[0][:], mult, add)
                    else:
                        nc.vector.tensor_scalar(
                            qA[64:128, nsl], ps[0:64, :], 0.125,
                            bqt[1][:], mult, add)
                        nc.vector.tensor_scalar(
                            qB[0:64, nsl], ps[64:128, :], 0.125,
                            bqt[2][:], mult, add)

                def v_chunk(m):
                    xh = xtf if m < 8 else xts
                    msl = slice(128 * (m % 8), 128 * (m % 8 + 1))
                    pf = ps2.tile([128, NBLK], f32, tag="s", bufs=3,
                                  name="ps_v")
                    ps = pf[:, 0:512]
                    for k in range(KC):
                        nc.tensor.matmul(ps[:, 0:J], xh[k][:, msl], wv[k][:],
                                         start=(k == 0), stop=(k == KC - 1))
                    vdst = vx[m].rearrange("p (h e) -> p h e",
                                           e=65)[:, :, 0:64]
                    nc.vector.tensor_copy(
                        vdst, ps[:, 0:J].rearrange("p (h e) -> p h e", e=64))

                pend = []

                def av_flush():
                    avh, h, nb, m, e = pend.pop(0)
                    vsl = slice(65 * h, 65 * (h + 1))
                    for i in range(2):
                        nc.tensor.matmul(
                            avh[i][:], vx[m][:, vsl],
                            e[:, 512 * i:512 * (i + 1)],
                            start=(m == 0), stop=(m == MC - 1))
                    if m != MC - 1:
                        return
                    adst, r0 = ((ah2[0], 0) if h == 0 else
                                (ah2[0], 64) if h == 1 else
                                (ah2[1], 0))
                    # raw copies first — they alone gate the next pair's
                    # AV PSUM writes; on ACT so the DVE exp stream is
                    # not disturbed. Sums rows and the broadcast/
                    # reciprocal/multiply chains follow off the critical
                    # path
                    for i in range(2):
                        hf = slice(NBLK * nb + 512 * i,
                                   NBLK * nb + 512 * (i + 1))
                        nc.scalar.activation(raw[h][:, hf],
                                             avh[i][0:64, :], Copy)
                    for i in range(2):
                        hf = slice(NBLK * nb + 512 * i,
                                   NBLK * nb + 512 * (i + 1))
                        nc.vector.tensor_copy(sums[h][:, hf],
                                              avh[i][64:65, :])
                    for i in range(2):
                        hf = slice(NBLK * nb + 512 * i,
                                   NBLK * nb + 512 * (i + 1))
                        bcs = pbc.tile([64, 512], f32, tag="bcs",
                                       name="bcs")
                        nc.gpsimd.partition_broadcast(bcs[:],
                                                      sums[h][:, hf])
                        rec = pbc.tile([64, 512], f32, tag="rec",
                                       name="rec")
                        nc.vector.reciprocal_approx_fast(rec[:], bcs[:])
                        nc.vector.tensor_mul(adst[r0:r0 + 64, hf],
                                             raw[h][:, hf], rec[:])

                def attn_pair(h, nb, extra=None):
                    # extra: dict chunk -> [thunks] fired after that
                    # chunk (interleaved projection / late front work)
                    qt = qA if h < 2 else qB
                    avh = [ps2.tile([65, 512], f32, tag=f"av{i}", bufs=1,
                                    name=f"ps_av{i}") for i in range(2)]
                    for m in range(MC):
                        msl = slice(128 * m, 128 * (m + 1))
                        s = ps2.tile([128, NBLK], f32, tag="s", bufs=3,
                                     name="ps_s")
                        for i in range(2):
                            q0 = NBLK * nb + 512 * i
                            nc.tensor.matmul(
                                s[:, 512 * i:512 * (i + 1)],
                                kh[h][:, msl], qt[:, q0:q0 + 512])
                        e = pe.tile([128, NBLK], bf16, tag="e", name="e")
                        if m % 2 == 1:
                            # odd chunks: Schraudolph exp on DVE
                            # (int16 bit trick, ~3% elementwise error
                            # that mostly cancels in softmax) to keep
                            # the ACT engine off the critical path
                            nc.vector.tensor_scalar(
                                e.bitcast(i16)[:], s[:],
                                SCH_A, SCH_B, mult, add)
                        else:
                            nc.scalar.activation(e[:], s[:], Exp)
                        pend.append((avh, h, nb, m, e))
                        if len(pend) > 6:
                            av_flush()
                        if extra and m in extra:
                            for th in extra[m]:
                                th()

                def proj(m):
                    # output projection chunk, borrowing an "s" PSUM
                    # slot so it can interleave into attention pairs;
                    # PSUM evacuation split across ACT and DVE
                    msl = slice(128 * m, 128 * (m + 1))
                    proj1(m)
                    proj2(m)

                pjt = {}

                def proj1(m):
                    # ah2[1] (head 2) part first: it is flushed two
                    # pairs before ah2[0], so these matmuls can run
                    # while the last pair's normalize chain completes
                    msl = slice(128 * m, 128 * (m + 1))
                    t = ps2.tile([128, NBLK], f32, tag="s", bufs=3,
                                 name="ps_pj")
                    pjt[m] = t
                    nc.tensor.matmul(t[:, 0:512], ah2[1][:, msl],
                                     wp[1][:, 0:512],
                                     start=True, stop=False)
                    nc.tensor.matmul(t[:, 512:768], ah2[1][:, msl],
                                     wp[1][:, 512:768],
                                     start=True, stop=False)

                def proj2(m):
                    msl = slice(128 * m, 128 * (m + 1))
                    t = pjt.pop(m)
                    nc.tensor.matmul(t[:, 0:512], ah2[0][:, msl],
                                     wp[0][:, 0:512],
                                     start=False, stop=True)
                    nc.tensor.matmul(t[:, 512:768], ah2[0][:, msl],
                                     wp[0][:, 512:768],
                                     start=False, stop=True)
                    o3 = posb.tile([128, C], bf16, tag="o3", name="o3")
                    nc.vector.tensor_copy(o3[:, 0:512], t[:, 0:512])
                    nc.scalar.activation(o3[:, 512:768], t[:, 512:768],
                                         Copy)
                    # rotate output queues so the final DMA drain
                    # is not serialized on one ring
                    oq = [nc.sync, nc.scalar, nc.gpsimd][m % 3]
                    oq.dma_start(out_d[msl, :], o3[:])

                def gg(b, g):
                    return lambda: g_group(b, g)

                def vv(m):
                    return lambda: v_chunk(m)

                def pj(m):
                    return lambda: proj(m)

                # minimal pre-pair front: only what pair (2,0) chunks
                # 0..7 need (kh2 + qB over the first x^T halves).
                # Everything else - kh0/kh1, b2/b3 projections, all V
                # chunks - rides inside pair (2,0), overlapping the
                # second x^T halves still streaming in. Block-0
                # output projection rides inside the block-1 pairs,
                # so only 8 proj chunks and the last flush remain in
                # the serial tail.
                for b in (0, 1):
                    g_group(b, 1)
                for b in (0, 1):
                    g_group(b, 2)
                ext0 = {0: [vv(0), gg(0, 0)], 1: [vv(1), gg(1, 0)],
                        2: [vv(2)], 3: [vv(3)],
                        4: [vv(4), gg(2, 1)], 5: [vv(5), gg(3, 1)],
                        6: [vv(6), gg(2, 0)], 7: [vv(7), gg(3, 0)],
                        8: [vv(8), gg(2, 2)], 9: [vv(9), gg(3, 2)]}
                for m in range(10, MC):
                    ext0[m] = [vv(m)]
                attn_pair(2, 0, ext0)
                attn_pair(0, 0)
                attn_pair(1, 0)
                # block-0 proj spread thinly over all block-1 pairs
                # ((2,1) only from chunk 8, after (1,0)'s normalize)
                attn_pair(2, 1, {8: [pj(0)], 11: [pj(1)]})
                attn_pair(0, 1, {4: [pj(2)], 8: [pj(3)], 12: [pj(4)]})
                attn_pair(1, 1, {4: [pj(5)], 8: [pj(6)], 12: [pj(7)]})
                while pend:
                    av_flush()
                # tail: run the ah2[1] halves of the first three
                # proj chunks under the last flush's normalize chain,
                # then pipeline phase1/phase2
                for m in range(8, 11):
                    proj1(m)
                for m in range(8, MC):
                    proj2(m)
                    if m + 3 < MC:
                        proj1(m + 3)

    nc.compile()
    return nc


def kernel(x, w_qkv, b_qkv, w_proj, b_proj):
    import ml_dtypes

    from concourse.bass_utils import run_bass_kernel_spmd

    global LAST_RESULTS
    if "nc" not in _cache:
        _cache["nc"] = _build()
    nc = _cache["nc"]

    bf16 = ml_dtypes.bfloat16
    x = np.asarray(x, dtype=np.float32)
    w_qkv = np.asarray(w_qkv, dtype=np.float32)
    b_qkv = np.asarray(b_qkv, dtype=np.float32)
    w_proj = np.asarray(w_proj, dtype=np.float32)
    b_proj = np.asarray(b_proj, dtype=np.float32)

    in_maps = []
    for c in range(NCORES):
        b = c // 4
        h0 = HPC * (c % 4)
        qs = slice(64 * h0, 64 * (h0 + HPC))
        ks = slice(C + 64 * h0, C + 64 * (h0 + HPC))
        vs = slice(2 * C + 64 * h0, 2 * C + 64 * (h0 + HPC))
        wkq = np.concatenate([w_qkv[:, ks], w_qkv[:, qs]], axis=1)
        wp_pad = np.zeros((2 * 128, C), dtype=np.float32)
        wp_pad[0:128] = w_proj[64 * h0:64 * (h0 + 2), :]
        wp_pad[128:192] = w_proj[64 * (h0 + 2):64 * (h0 + 3), :]
        in_maps.append({
            "xt": np.ascontiguousarray(x[b].T).astype(bf16),
            "wkq": np.ascontiguousarray(wkq).astype(bf16),
            "wv": np.ascontiguousarray(w_qkv[:, vs]).astype(bf16),
            "bq": np.ascontiguousarray(
                (b_qkv[qs] * 0.125).reshape(J, 1)).astype(np.float32),
            "wp": wp_pad.astype(bf16),
        })

    res = run_bass_kernel_spmd(nc, in_maps, core_ids=list(range(NCORES)))
    LAST_RESULTS = res

    out = np.zeros((B, N, C), dtype=np.float32)
    for c in range(NCORES):
        out[c // 4] += np.asarray(res.results[c]["out"], dtype=np.float32)
    out += b_proj + b_qkv[2 * C:] @ w_proj
    return out



# revision 66
# speedup vs baseline: 1.0562x; 1.0142x over previous
"""Multi-head attention block on 8 TRN2 NeuronCores.

Problem: x[2,2048,768] -> qkv proj -> 12-head attention -> out proj.
Sharding: 24 (batch, head) pairs across 8 cores; core c handles batch
c//4 and heads 3*(c%4)..3*(c%4)+2. Each core computes its heads'
Q,K,V, attention, and a partial output projection; the host sums the
four per-batch partials and adds the bias terms.

Design notes (v9, ~163.5us vs v6's ~176us):
  - Exp split across engines: odd key chunks compute exp on the DVE
    via a Schraudolph bit trick (one tensor_scalar:
    int16(s*184.665 + 16249) bitcast to bf16; the ~3% elementwise
    error mostly cancels in softmax, rel err 1.07e-2 vs the 2e-2
    gate). This halves the ACT exp stream, making the attention
    inner loop PE-bound (~98% TensorE occupancy) instead of
    ACT-bound. Pair-end raw copies move to ACT for the same reason.
  - One PSUM pool for the whole kernel: s tiles (3 bufs, 6 banks) +
    two AV accumulators (2 banks). Front work (QKV projections, V
    chunks) and the output projection borrow s slots, so they can
    interleave anywhere in the attention stream.
  - Deep AV deferral: the AV matmul for chunk m runs ~7 chunks
    later (12 e-tile buffers). This soaks up pair-boundary and
    flush-chain jitter that otherwise stalls the PE ~1us per pair;
    gains were monotone in the lag until ~7 (each step needs the
    matching e-buffer count - at 4 buffers the same lag REGRESSES
    because exp stalls on e-slot recycling).
  - Minimal pre-pair front: only kh2/qB over the first x^T halves
    runs before attention; kh0/kh1, the b2/b3 projections and all V
    chunks ride inside pair (2,0) while the second x^T halves
    stream in (separate first/second-half x^T tiles keep the
    dependencies decoupled). Attention starts ~33us in (was ~47us).
  - Pair order (2,0),(0,0),(1,0),(2,1),(0,1),(1,1); the block-0
    output projection rides inside the block-1 pairs, and the tail
    runs the ah2[1] halves of the first three remaining proj chunks
    under the last flush's normalize chain (proj accumulates head 2
    first for this reason). Output DMA alternates sync/scalar
    queues so the final drain is not serialized.
  - All matmul operands bf16; output bf16; host sums partials in
    f32. K/Q fused weight stream, packed Q tiles, zero-carrying K
    tiles, ones-column V tiles for the softmax sums, DVE-memset
    constants and PE warmup matmuls carried over from v6.
  - Things measured NOT to help on this hardware: tile_position row
    tiling for the 64-deep QK contraction (no MM concurrency, and
    half-array matmuls re-throttle the HAM clock gate to 1.2GHz),
    1024-column moving matmuls (ISA caps at 512), xt DMA on the
    gpsimd queue, gpsimd tensor ops beyond partition_broadcast
    (multi-us library reloads), fp8 DoubleRow AV (needs fp8 exp
    output, which blows the error budget).
"""

import os
import sys

for _p in ("/opt/trn_rl_repo", "/opt/pypackages"):
    if _p not in sys.path:
        sys.path.append(_p)

import numpy as np

B, N, C = 2, 2048, 768
H, D = 12, 64
HPC = 3                    # heads per core
J = HPC * D                # 192 per-core head-dim rows
NCORES = 8
NBLK = 1024                # query-block width (one exp per [128, NBLK])
NB = N // NBLK             # 2
MC = N // 128              # 16 key chunks
KC = C // 128              # 6 contraction chunks for projections
NWARM = 12

SCH_A = 184.6649652337873   # 2^7 / ln 2
SCH_B = 16249.0             # exponent-bias offset, tuned for softmax

_cache = {}
LAST_RESULTS = None


def _build():
    import concourse.mybir as mybir
    import concourse.tile as tile
    from concourse import bacc

    f32 = mybir.dt.float32
    bf16 = mybir.dt.bfloat16
    i16 = mybir.dt.int16
    Exp = mybir.ActivationFunctionType.Exp
    Copy = mybir.ActivationFunctionType.Copy
    mult = mybir.AluOpType.mult
    add = mybir.AluOpType.add

    nc = bacc.Bacc("TRN2", target_bir_lowering=False, debug=False,
                   num_devices=NCORES)

    xt_d = nc.declare_dram_parameter("xt", [C, N], bf16, isOutput=False)
    # fused [K_h0|K_h1|K_h2|Q_h0|Q_h1|Q_h2] weight columns
    wkq_d = nc.declare_dram_parameter("wkq", [C, 2 * J], bf16,
                                      isOutput=False)
    wv_d = nc.declare_dram_parameter("wv", [C, J], bf16, isOutput=False)
    bq_d = nc.declare_dram_parameter("bq", [J, 1], f32, isOutput=False)
    # padded proj weights: rows 0:128 = heads 0,1; 128:192 = head 2;
    # 192:256 = zero (annihilates ah2[1]'s junk bottom half)
    wp_d = nc.declare_dram_parameter("wp", [2 * 128, C], bf16,
                                     isOutput=False)
    out_d = nc.declare_dram_parameter("out", [N, C], bf16, isOutput=True)

    with tile.TileContext(nc) as tc:
        with (
            tc.tile_pool(name="persist", bufs=1) as pp,
            tc.tile_pool(name="osb", bufs=4) as posb,
            tc.tile_pool(name="etile", bufs=12) as pe,
            tc.tile_pool(name="bcsb", bufs=2) as pbc,
        ):
            warm_t = pp.tile([128, 512], bf16, tag="warm_t", name="warm_t")
            wkq = [pp.tile([128, 2 * J], bf16, tag=f"wkq{k}",
                           name=f"wkq{k}") for k in range(KC)]
            # x^T halves as separate tiles so first-half
            # consumers never wait on the second-half DMA
            xtf = [pp.tile([128, 1024], bf16, tag=f"xtf{k}",
                           name=f"xtf{k}") for k in range(KC)]
            xts = [pp.tile([128, 1024], bf16, tag=f"xts{k}",
                           name=f"xts{k}") for k in range(KC)]
            wv = [pp.tile([128, J], bf16, tag=f"wv{k}", name=f"wv{k}")
                  for k in range(KC)]
            bqt = [pp.tile([64, 1], f32, tag=f"bq{h}", name=f"bq{h}")
                   for h in range(HPC)]
            wp = [pp.tile([128, C], bf16, tag=f"wp{t}", name=f"wp{t}")
                  for t in range(2)]
            # K^T per head, zero rows alternating so the packed Q tiles
            # need none: kh0 data 0:64, kh1 data 64:128, kh2 data 0:64
            kh = [pp.tile([128, N], bf16, tag=f"kh{h}", name=f"kh{h}")
                  for h in range(HPC)]
            # Q^T packed: qA = (q0 top, q1 bottom); qB = (q2 top, junk)
            qA = pp.tile([128, N], bf16, tag="qA", name="qA")
            qB = pp.tile([128, N], bf16, tag="qB", name="qB")
            # V with a ones column per head: [128, 3*65]
            vx = [pp.tile([128, HPC * 65], bf16, tag=f"vx{m}",
                          name=f"vx{m}") for m in range(MC)]
            sums = [pp.tile([1, N], f32, tag=f"sums{h}", name=f"sums{h}")
                    for h in range(HPC)]
            # unnormalized attention outputs (release AV PSUM fast),
            # base partition 0 so tensor_mul's SBUF inputs stay aligned
            raw = [pp.tile([64, N], bf16, tag=f"raw{h}", name=f"raw{h}")
                   for h in range(HPC)]
            ah2 = [pp.tile([128, N], bf16, tag=f"ah2{t}", name=f"ah2{t}")
                   for t in range(2)]

            # ---- constants via DVE memset (no DMA traffic; gpsimd
            # memset on partition-offset APs crashed the device) ----
            nc.vector.memset(warm_t[:], 1.0)
            nc.vector.memset(kh[0][64:128, :], 0.0)
            nc.vector.memset(kh[1][0:64, :], 0.0)
            nc.vector.memset(kh[2][64:128, :], 0.0)
            nc.vector.memset(qB[64:128, :], 0.0)
            nc.vector.memset(ah2[1][64:128, :], 0.0)
            for m in range(MC):
                on = vx[m].rearrange("p (h e) -> p h e", e=65)[:, :, 64:65]
                nc.vector.memset(on, 1.0)

            # ---- input DMA, ordered for earliest compute start ----
            # sync queue: x^T half-row tiles only (the long pole; each
            # extra trigger ahead of an xt tile delays it ~0.6us), first
            # halves feed blocks 0/1, then second halves
            for k in range(KC):
                nc.sync.dma_start(xtf[k][:],
                                  xt_d[128 * k:128 * (k + 1), 0:1024])
            for k in range(KC):
                nc.sync.dma_start(xts[k][:],
                                  xt_d[128 * k:128 * (k + 1), 1024:2048])
            # scalar queue (ACT idle early): fused KQ weights first
            # (k-th tile lands just ahead of the k-th accumulation
            # step; the tiny bias tiles would delay wkq[0] ~0.6us per
            # trigger), then q-bias, then V weights
            for k in range(KC):
                nc.scalar.dma_start(wkq[k][:], wkq_d[128 * k:128 * (k + 1), :])
            for h in range(HPC):
                nc.scalar.dma_start(bqt[h][:], bq_d[64 * h:64 * (h + 1), :])
            for k in range(KC):
                nc.scalar.dma_start(wv[k][:], wv_d[128 * k:128 * (k + 1), :])

            # wp is only needed by the projection tail; queue it after
            # the x^T burst
            for t in range(2):
                nc.sync.dma_start(wp[t][:], wp_d[128 * t:128 * (t + 1), :])

            with tc.tile_pool(name="ps2", bufs=1, space="PSUM") as ps2:
                # all front work (warmup, QKV projections, V chunks)
                # borrows "s" PSUM slots, so half of it can interleave
                # into the first attention pair while the second x^T
                # halves are still streaming in
                for i in range(NWARM):
                    ps = ps2.tile([128, NBLK], f32, tag="s", bufs=3,
                                  name=f"warm{i}")
                    nc.tensor.matmul(ps[:, 0:512], warm_t[:, 0:128],
                                     warm_t[:])

                def g_group(b, g):
                    # g0: K_h0,K_h1 / g1: K_h2,Q_h0 / g2: Q_h1,Q_h2
                    nsl = slice(512 * b, 512 * (b + 1))
                    xh = xtf if b < 2 else xts
                    hsl = slice(512 * (b % 2), 512 * (b % 2 + 1))
                    pf = ps2.tile([128, NBLK], f32, tag="s", bufs=3,
                                  name="ps_qk")
                    ps = pf[:, 0:512]
                    for k in range(KC):
                        nc.tensor.matmul(
                            ps, wkq[k][:, 128 * g:128 * (g + 1)],
                            xh[k][:, hsl],
                            start=(k == 0), stop=(k == KC - 1))
                    if g == 0:
                        nc.vector.tensor_copy(kh[0][0:64, nsl], ps[0:64, :])
                        nc.vector.tensor_copy(kh[1][64:128, nsl],
                                              ps[64:128, :])
                    elif g == 1:
                        nc.vector.tensor_copy(kh[2][0:64, nsl], ps[0:64, :])
                        nc.vector.tensor_scalar(
                            qA[0:64, nsl], ps[64:128, :], 0.125,
                            bqt[0][:], mult, add)
                    else:
                        nc.vector.tensor_scalar(
                            qA[64:128, nsl], ps[0:64, :], 0.125,
                            bqt[1][:], mult, add)
                        nc.vector.tensor_scalar(
                            qB[0:64, nsl], ps[64:128, :], 0.125,
                            bqt[2][:], mult, add)

                def v_chunk(m):
                    xh = xtf if m < 8 else xts
                    msl = slice(128 * (m % 8), 128 * (m % 8 + 1))
                    pf = ps2.tile([128, NBLK], f32, tag="s", bufs=3,
                                  name="ps_v")
                    ps = pf[:, 0:512]
                    for k in range(KC):
                        nc.tensor.matmul(ps[:, 0:J], xh[k][:, msl], wv[k][:],
                                         start=(k == 0), stop=(k == KC - 1))
                    vdst = vx[m].rearrange("p (h e) -> p h e",
                                           e=65)[:, :, 0:64]
                    nc.vector.tensor_copy(
                        vdst, ps[:, 0:J].rearrange("p (h e) -> p h e", e=64))

                pend = []

                def av_flush():
                    avh, h, nb, m, e = pend.pop(0)
                    vsl = slice(65 * h, 65 * (h + 1))
                    for i in range(2):
                        nc.tensor.matmul(
                            avh[i][:], vx[m][:, vsl],
                            e[:, 512 * i:512 * (i + 1)],
                            start=(m == 0), stop=(m == MC - 1))
                    if m != MC - 1:
                        return
                    adst, r0 = ((ah2[0], 0) if h == 0 else
                                (ah2[0], 64) if h == 1 else
                                (ah2[1], 0))
                    # raw copies first — they alone gate the next pair's
                    # AV PSUM writes; on ACT so the DVE exp stream is
                    # not disturbed. Sums rows and the broadcast/
                    # reciprocal/multiply chains follow off the critical
                    # path
                    for i in range(2):
                        hf = slice(NBLK * nb + 512 * i,
                                   NBLK * nb + 512 * (i + 1))
                        nc.scalar.activation(raw[h][:, hf],
                                             avh[i][0:64, :], Copy)
                    for i in range(2):
                        hf = slice(NBLK * nb + 512 * i,
                                   NBLK * nb + 512 * (i + 1))
                        nc.vector.tensor_copy(sums[h][:, hf],
                                              avh[i][64:65, :])
                    for i in range(2):
                        hf = slice(NBLK * nb + 512 * i,
                                   NBLK * nb + 512 * (i + 1))
                        bcs = pbc.tile([64, 512], f32, tag="bcs",
                                       name="bcs")
                        nc.gpsimd.partition_broadcast(bcs[:],
                                                      sums[h][:, hf])
                        rec = pbc.tile([64, 512], f32, tag="rec",
                                       name="rec")
                        nc.vector.reciprocal_approx_fast(rec[:], bcs[:])
                        nc.vector.tensor_mul(adst[r0:r0 + 64, hf],
                                             raw[h][:, hf], rec[:])

                def attn_pair(h, nb, extra=None):
                    # extra: dict chunk -> [thunks] fired after that
                    # chunk (interleaved projection / late front work)
                    qt = qA if h < 2 else qB
                    avh = [ps2.tile([65, 512], f32, tag=f"av{i}", bufs=1,
                                    name=f"ps_av{i}") for i in range(2)]
                    for m in range(MC):
                        msl = slice(128 * m, 128 * (m + 1))
                        s = ps2.tile([128, NBLK], f32, tag="s", bufs=3,
                                     name="ps_s")
                        for i in range(2):
                            q0 = NBLK * nb + 512 * i
                            nc.tensor.matmul(
                                s[:, 512 * i:512 * (i + 1)],
                                kh[h][:, msl], qt[:, q0:q0 + 512])
                        e = pe.tile([128, NBLK], bf16, tag="e", name="e")
                        if m % 2 == 1:
                            # odd chunks: Schraudolph exp on DVE
                            # (int16 bit trick, ~3% elementwise error
                            # that mostly cancels in softmax) to keep
                            # the ACT engine off the critical path
                            nc.vector.tensor_scalar(
                                e.bitcast(i16)[:], s[:],
                                SCH_A, SCH_B, mult, add)
                        else:
                            nc.scalar.activation(e[:], s[:], Exp)
                        pend.append((avh, h, nb, m, e))
                        if len(pend) > 6:
                            av_flush()
                        if extra and m in extra:
                            for th in extra[m]:
                                th()

                def proj(m):
                    # output projection chunk, borrowing an "s" PSUM
                    # slot so it can interleave into attention pairs;
                    # PSUM evacuation split across ACT and DVE
                    msl = slice(128 * m, 128 * (m + 1))
                    proj1(m)
                    proj2(m)

                pjt = {}

                def proj1(m):
                    # ah2[1] (head 2) part first: it is flushed two
                    # pairs before ah2[0], so these matmuls can run
                    # while the last pair's normalize chain completes
                    msl = slice(128 * m, 128 * (m + 1))
                    t = ps2.tile([128, NBLK], f32, tag="s", bufs=3,
                                 name="ps_pj")
                    pjt[m] = t
                    nc.tensor.matmul(t[:, 0:512], ah2[1][:, msl],
                                     wp[1][:, 0:512],
                                     start=True, stop=False)
                    nc.tensor.matmul(t[:, 512:768], ah2[1][:, msl],
                                     wp[1][:, 512:768],
                                     start=True, stop=False)

                def proj2(m):
                    msl = slice(128 * m, 128 * (m + 1))
                    t = pjt.pop(m)
                    nc.tensor.matmul(t[:, 0:512], ah2[0][:, msl],
                                     wp[0][:, 0:512],
                                     start=False, stop=True)
                    nc.tensor.matmul(t[:, 512:768], ah2[0][:, msl],
                                     wp[0][:, 512:768],
                                     start=False, stop=True)
                    o3 = posb.tile([128, C], bf16, tag="o3", name="o3")
                    nc.vector.tensor_copy(o3[:, 0:512], t[:, 0:512])
                    nc.scalar.activation(o3[:, 512:768], t[:, 512:768],
                                         Copy)
                    # rotate output queues so the final DMA drain
                    # is not serialized on one ring
                    oq = [nc.sync, nc.scalar, nc.gpsimd][m % 3]
                    oq.dma_start(out_d[msl, :], o3[:])

                def gg(b, g):
                    return lambda: g_group(b, g)

                def vv(m):
                    return lambda: v_chunk(m)

                def pj(m):
                    return lambda: proj(m)

                # minimal pre-pair front: only what pair (2,0) chunks
                # 0..7 need (kh2 + qB over the first x^T halves).
                # Everything else - kh0/kh1, b2/b3 projections, all V
                # chunks - rides inside pair (2,0), overlapping the
                # second x^T halves still streaming in. Block-0
                # output projection rides inside the block-1 pairs,
                # so only 8 proj chunks and the last flush remain in
                # the serial tail.
                for b in (0, 1):
                    g_group(b, 1)
                for b in (0, 1):
                    g_group(b, 2)
                ext0 = {0: [vv(0), gg(0, 0)], 1: [vv(1), gg(1, 0)],
                        2: [vv(2)], 3: [vv(3)],
                        4: [vv(4), gg(2, 1)], 5: [vv(5), gg(3, 1)],
                        6: [vv(6), gg(2, 0)], 7: [vv(7), gg(3, 0)],
                        8: [vv(8), gg(2, 2)], 9: [vv(9), gg(3, 2)]}
                for m in range(10, MC):
                    ext0[m] = [vv(m)]
                attn_pair(2, 0, ext0)
                attn_pair(0, 0)
                attn_pair(1, 0)
                # block-0 proj spread thinly over all block-1 pairs
                # ((2,1) only from chunk 8, after (1,0)'s normalize)
                attn_pair(2, 1, {8: [pj(0)], 11: [pj(1)]})
                attn_pair(0, 1, {4: [pj(2)], 8: [pj(3)], 12: [pj(4)]})
                attn_pair(1, 1, {4: [pj(5)], 8: [pj(6)], 12: [pj(7)]})
                while pend:
                    av_flush()
                # tail: run the ah2[1] halves of the first three
                # proj chunks under the last flush's normalize chain,
                # then pipeline phase1/phase2
                for m in range(8, 11):
                    proj1(m)
                for m in range(8, MC):
                    proj2(m)
                    if m + 3 < MC:
                        proj1(m + 3)

    nc.compile()
    return nc


def kernel(x, w_qkv, b_qkv, w_proj, b_proj):
    import ml_dtypes

    from concourse.bass_utils import run_bass_kernel_spmd

    global LAST_RESULTS
    if "nc" not in _cache:
        _cache["nc"] = _build()
    nc = _cache["nc"]

    bf16 = ml_dtypes.bfloat16
    x = np.asarray(x, dtype=np.float32)
    w_qkv = np.asarray(w_qkv, dtype=np.float32)
    b_qkv = np.asarray(b_qkv, dtype=np.float32)
    w_proj = np.asarray(w_proj, dtype=np.float32)
    b_proj = np.asarray(b_proj, dtype=np.float32)

    in_maps = []
    for c in range(NCORES):
        b = c // 4
        h0 = HPC * (c % 4)
        qs = slice(64 * h0, 64 * (h0 + HPC))
        ks = slice(C + 64 * h0, C + 64 * (h0 + HPC))
        vs = slice(2 * C + 64 * h0, 2 * C + 64 * (h0 + HPC))
        wkq = np.concatenate([w_qkv[:, ks], w_qkv[:, qs]], axis=1)
        wp_pad = np.zeros((2 * 128, C), dtype=np.float32)
        wp_pad[0:128] = w_proj[64 * h0:64 * (h0 + 2), :]
        wp_pad[128:192] = w_proj[64 * (h0 + 2):64 * (h0 + 3), :]
        in_maps.append({
            "xt": np.ascontiguousarray(x[b].T).astype(bf16),
            "wkq": np.ascontiguousarray(wkq).astype(bf16),
            "wv": np.ascontiguousarray(w_qkv[:, vs]).astype(bf16),
            "bq": np.ascontiguousarray(
                (b_qkv[qs] * 0.125).reshape(J, 1)).astype(np.float32),
            "wp": wp_pad.astype(bf16),
        })

    res = run_bass_kernel_spmd(nc, in_maps, core_ids=list(range(NCORES)))
    LAST_RESULTS = res

    out = np.zeros((B, N, C), dtype=np.float32)
    for c in range(NCORES):
        out[c // 4] += np.asarray(res.results[c]["out"], dtype=np.float32)
    out += b_proj + b_qkv[2 * C:] @ w_proj
    return out

